# revision 5
# baseline (speedup 1.0000x reference)
"""GraphSAGE 2-block GNN (nn_BaselineModel_80607946211554) on 8 TRN2 NeuronCores.

Strategy: destination-node sharding, bf16 datapath. Each core owns 6250
contiguous nodes. Node-feature tables are replicated per-core in DRAM in a
slab layout (node n -> row (n//6250)*6272 + n%6250, 22 zero pad rows/slab).
Neighbor mean-aggregation per 128-dst window: dma_gather of bf16 source rows
(edges sorted by dst, host-preprocessed, sections padded to 128 only), then
for each 128-slot tile a DVE-generated indicator (iota==dloc)*inv_deg feeds a
PE matmul accumulating mean^T directly in PSUM. SAGE linears run
feature-major (weights stationary); PSUM->SBUF copies and bias+ReLU run on
the Activation engine. Intermediate tables rebuilt via bf16 AllGather; graph
pooling is a one-hot matmul; the MLP head + softmax is replicated per core.

Self-contained: hardcodes all shapes for the fixed problem instance.
"""
import os
import sys
import types
import numpy as np

N = 50000
E = 1600000
G = 256
F = 128
HID = 128
C = 10
NCORES = 8
NPC = N // NCORES            # 6250 nodes per core
SLAB = 6272                  # slab rows (6250 + 22 zero pad)
NT = NCORES * SLAB           # 50176 table rows
LO = 4 * SLAB                # 25088; table rows < LO hold nodes < 25000
PADROW = 6250                # zero row (local index in both lo/hi views)
P = 128
NW = (NPC + P - 1) // P      # 49 dst windows per core
EPS = 1e-5
GCAP = 168                   # max 128-slot tiles per gather group (5.5MB bf16)

_prog_cache = {}


def _bf16(a):
    import concourse.mybir as mybir
    return np.asarray(a, np.float32).astype(mybir.dt.np(mybir.dt.bfloat16))


def _wrap16(vals):
    """int64 slot values (len mult of 16) -> [128, n/16] int16 wrapped."""
    n = len(vals)
    arr = vals.reshape(n // 16, 16).T.astype(np.int16)   # [16, n/16]
    return np.tile(arr, (8, 1))                           # [128, n/16]


def _wrap128(vals):
    """[S] -> [128, S/128]: slot s -> [s%128, s//128]."""
    return vals.reshape(-1, 128).T.copy()


def _build_schedule(src, dst, invd_full):
    """Static shared schedule + per-core gather index / metadata arrays."""
    core_edges = []
    CL = np.zeros((NCORES, NW), np.int64)
    CH = np.zeros((NCORES, NW), np.int64)
    for c in range(NCORES):
        m = (dst >= c * NPC) & (dst < (c + 1) * NPC)
        s = src[m].astype(np.int64)
        d = (dst[m] - c * NPC).astype(np.int64)
        hi = (s >= N // 2).astype(np.int64)
        w = d >> 7
        order = np.lexsort((d, hi, w))
        s, d, hi, w = s[order], d[order], hi[order], w[order]
        core_edges.append((s, d, hi, w))
        cnt = np.bincount(w * 2 + hi, minlength=NW * 2).reshape(NW, 2)
        CL[c], CH[c] = cnt[:, 0], cnt[:, 1]

    nL = np.maximum(((CL.max(0) + 127) // P) * P, P)
    nH = np.maximum(((CH.max(0) + 127) // P) * P, P)
    gL, gH = nL // P, nH // P                     # tiles per section
    ngrp = gL + gH

    # pack consecutive windows into gather groups of <= GCAP tiles
    groups = []
    cur = []
    cur_cols = 0
    for w in range(NW):
        if cur and cur_cols + ngrp[w] > GCAP:
            groups.append(cur)
            cur, cur_cols = [], 0
        cur.append(w)
        cur_cols += int(ngrp[w])
    if cur:
        groups.append(cur)

    # layouts
    ginfo = []       # per group: dict
    slot_base = np.zeros(NW, np.int64)   # base slot of lo section of window
    hslot_base = np.zeros(NW, np.int64)  # base slot of hi section of window
    col0 = 0         # running tile column over all groups
    for ws in groups:
        colsL = int(gL[ws].sum())
        cols = int(ngrp[ws].sum())
        off = 0
        for w in ws:
            slot_base[w] = (col0 + off) * P
            off += int(gL[w])
        for w in ws:
            hslot_base[w] = (col0 + off) * P
            off += int(gH[w])
        ginfo.append(dict(ws=ws, colsL=colsL, cols=cols, col0=col0))
        col0 += cols
    tot_cols = col0
    S_tot = tot_cols * P

    sched = dict(nL=nL, nH=nH, gL=gL, gH=gH, ngrp=ngrp, groups=ginfo,
                 slot_base=slot_base, hslot_base=hslot_base,
                 tot_cols=tot_cols, S_tot=S_tot,
                 gmax=max(g["cols"] for g in ginfo))

    per_core = []
    for c in range(NCORES):
        s, d, hi, w = core_edges[c]
        # rank within (w, hi) section
        key = w * 2 + hi
        if len(key):
            grp_change = np.r_[True, key[1:] != key[:-1]]
            first_pos = np.flatnonzero(grp_change)
            gidx = np.cumsum(grp_change) - 1
            rank = np.arange(len(d)) - first_pos[gidx]
        else:
            rank = np.zeros(0, np.int64)
        base = np.where(hi == 1, hslot_base[w], slot_base[w])
        pos = base + rank

        trow = (s // NPC) * SLAB + s % NPC
        tval = np.where(hi == 1, trow - LO, trow)

        idx_vals = np.full(S_tot, PADROW, np.int64)
        idx_vals[pos] = tval
        ind = np.zeros((S_tot, P), np.float32)
        ind[pos, d & 127] = invd_full[c * NPC + d]

        per_core.append(dict(
            idx=_wrap16(idx_vals),
            ind=_bf16(ind),
        ))
    return sched, per_core


def _host_inputs(inputs):
    import concourse.mybir as mybir
    bfnp = mybir.dt.np(mybir.dt.bfloat16)
    f32 = lambda a: np.asarray(a, np.float32)
    x = f32(inputs["x"])
    ei = np.asarray(inputs["edge_index"], np.int64)
    batch = np.asarray(inputs["batch"], np.int64)
    src, dst = ei[0], ei[1]

    deg = np.bincount(dst, minlength=N).astype(np.float32)
    invd_full = (1.0 / np.maximum(deg, 1.0)).astype(np.float32)

    sched, per_core = _build_schedule(src, dst, invd_full)

    xt = np.zeros((NT, F), bfnp)
    xb = _bf16(x)
    for r in range(NCORES):
        xt[r * SLAB:r * SLAB + NPC] = xb[r * NPC:(r + 1) * NPC]

    ident = np.eye(P, dtype=np.float32)

    # BN folding
    s_bn = f32(inputs["bn_gamma"]) / np.sqrt(f32(inputs["bn_rv"]) + EPS)
    t_bn = f32(inputs["bn_beta"]) - f32(inputs["bn_rm"]) * s_bn
    bns2 = s_bn.reshape(2, P).T.copy()     # [128, 2]
    bnt2 = t_bn.reshape(2, P).T.copy()

    shared = {
        "xt": xt, "ident": _bf16(ident),
        "bns2": bns2, "bnt2": bnt2,
        "l1w": _bf16(inputs["lin1_W"]), "l1b": f32(inputs["lin1_b"]),
        "l2w": _bf16(inputs["lin2_W"]), "l2b": f32(inputs["lin2_b"]),
    }
    for b in (0, 1):
        for nm in ("Wl1", "Wr1", "Wl2", "Wr2", "Wlin"):
            shared[f"b{b}_{nm}"] = _bf16(inputs[f"b{b}_{nm}"])
        for nm in ("b1", "b2", "blin"):
            shared[f"b{b}_{nm}"] = f32(inputs[f"b{b}_{nm}"])

    in_maps = []
    for c in range(NCORES):
        xoT = np.zeros((F, SLAB), bfnp)
        xoT[:, :NPC] = xb[c * NPC:(c + 1) * NPC].T
        pool_ind = np.zeros((NW, P, G), np.float32)
        bt = batch[c * NPC:(c + 1) * NPC]
        btp = np.full(NW * P, -1, np.int64)
        btp[:NPC] = bt
        btp2 = btp.reshape(NW, P)
        for wi in range(NW):
            vm = btp2[wi] >= 0
            pool_ind[wi, np.arange(P)[vm], btp2[wi][vm]] = 1.0
        im = dict(shared)
        im.update({
            "xoT": xoT, "poolind": _bf16(pool_ind),
            "idx": per_core[c]["idx"], "ind": per_core[c]["ind"],
        })
        in_maps.append(im)
    return sched, in_maps


# ------------------------------------------------------------- bass program
def _build_program(sched, n_convs=4, debug_tables=False):
    import concourse.bass as bass
    import concourse.mybir as mybir
    import concourse.tile as tile
    from concourse import bacc
    from concourse import library_config
    from contextlib import ExitStack

    dt = mybir.dt
    DT = dt.float32
    BF = dt.bfloat16
    Alu = mybir.AluOpType
    Act = mybir.ActivationFunctionType

    nL, nH, gL, gH = (sched[k] for k in ("nL", "nH", "gL", "gH"))
    groups = sched["groups"]
    slot_base, hslot_base = sched["slot_base"], sched["hslot_base"]
    GMAX = sched["gmax"]
    TOTC = sched["tot_cols"]

    nc = bacc.Bacc("TRN2", debug=False, num_swdge_queues=4)

    # ---- parameters
    xt = nc.declare_dram_parameter("xt", [NT, F], BF, isOutput=False)
    xoT = nc.declare_dram_parameter("xoT", [F, SLAB], BF, isOutput=False)
    idxp = nc.declare_dram_parameter("idx", [P, TOTC * 8], dt.int16, isOutput=False)
    indp = nc.declare_dram_parameter("ind", [TOTC * P, P], BF, isOutput=False)
    poolp = nc.declare_dram_parameter("poolind", [NW, P, G], BF, isOutput=False)
    identp = nc.declare_dram_parameter("ident", [P, P], BF, isOutput=False)
    wp = {}
    for b in (0, 1):
        for nm, shp, dty in (("Wl1", [F, HID], BF), ("Wr1", [F, HID], BF),
                             ("b1", [HID], DT),
                             ("Wl2", [HID, HID], BF), ("Wr2", [HID, HID], BF),
                             ("b2", [HID], DT),
                             ("Wlin", [2 * HID, HID], BF), ("blin", [HID], DT)):
            wp[f"b{b}_{nm}"] = nc.declare_dram_parameter(f"b{b}_{nm}", shp, dty, isOutput=False)
    bns2p = nc.declare_dram_parameter("bns2", [P, 2], DT, isOutput=False)
    bnt2p = nc.declare_dram_parameter("bnt2", [P, 2], DT, isOutput=False)
    l1wp = nc.declare_dram_parameter("l1w", [2 * HID, HID], BF, isOutput=False)
    l1bp = nc.declare_dram_parameter("l1b", [HID], DT, isOutput=False)
    l2wp = nc.declare_dram_parameter("l2w", [HID, C], BF, isOutput=False)
    l2bp = nc.declare_dram_parameter("l2b", [C], DT, isOutput=False)

    out = nc.declare_dram_parameter("out", [G, C], DT, isOutput=True)
    if debug_tables:
        dbgA = nc.declare_dram_parameter("dbgA", [NT, F], DT, isOutput=True)
        dbgB = nc.declare_dram_parameter("dbgB", [NT, F], DT, isOutput=True)

    with tile.TileContext(nc) as tc, ExitStack() as ctx:
        sb = ctx.enter_context(tc.tile_pool(name="sb", bufs=1))
        sb_feat = ctx.enter_context(tc.tile_pool(name="sb_feat", bufs=1))
        sb_g = ctx.enter_context(tc.tile_pool(name="sb_g", bufs=2))
        sb_idx = ctx.enter_context(tc.tile_pool(name="sb_idx", bufs=2))
        sb_ind = ctx.enter_context(tc.tile_pool(name="sb_ind", bufs=8))
        sb_ms = ctx.enter_context(tc.tile_pool(name="sb_ms", bufs=4))
        sb_pi = ctx.enter_context(tc.tile_pool(name="sb_pi", bufs=3))
        ps_agg = ctx.enter_context(tc.tile_pool(name="ps_agg", bufs=2, space="PSUM"))
        ps_mm = ctx.enter_context(tc.tile_pool(name="ps_mm", bufs=2, space="PSUM"))
        ps_tr = ctx.enter_context(tc.tile_pool(name="ps_tr", bufs=2, space="PSUM"))
        ps_pool = ctx.enter_context(tc.tile_pool(name="ps_pool", bufs=1, space="PSUM"))
        dram = ctx.enter_context(tc.tile_pool(name="dram", bufs=1, space="DRAM"))

        nc.gpsimd.load_library(library_config.mlp)

        # ---- constants into SBUF
        id_t = sb.tile([P, P], BF)
        nc.sync.dma_start(id_t[:], identp[:])
        wt = {}
        for b in (0, 1):
            for nm in ("Wl1", "Wr1", "Wl2", "Wr2"):
                w_t = sb.tile([P, P], BF, name=f"w{b}{nm}")
                nc.sync.dma_start(w_t[:], wp[f"b{b}_{nm}"][:])
                wt[f"b{b}_{nm}"] = w_t
            wlin_t = sb.tile([P, 2, P], BF, name=f"w{b}lin")
            nc.sync.dma_start(wlin_t[:, 0, :], wp[f"b{b}_Wlin"][0:P, :])
            nc.sync.dma_start(wlin_t[:, 1, :], wp[f"b{b}_Wlin"][P:2 * P, :])
            wt[f"b{b}_Wlin"] = wlin_t
            for nm in ("b1", "b2", "blin"):
                b_t = sb.tile([P, 1], DT, name=f"b{b}{nm}")
                nc.sync.dma_start(b_t[:], wp[f"b{b}_{nm}"][:, None])
                wt[f"b{b}_{nm}"] = b_t
        bns_t = sb.tile([P, 2], DT)
        nc.sync.dma_start(bns_t[:], bns2p[:])
        bnt_t = sb.tile([P, 2], DT)
        nc.sync.dma_start(bnt_t[:], bnt2p[:])
        l1w_t = sb.tile([P, 2, P], BF)
        nc.sync.dma_start(l1w_t[:, 0, :], l1wp[0:P, :])
        nc.sync.dma_start(l1w_t[:, 1, :], l1wp[P:2 * P, :])
        l1b_t = sb.tile([P, 1], DT)
        nc.sync.dma_start(l1b_t[:], l1bp[:, None])
        l2w_t = sb.tile([P, C], BF)
        nc.sync.dma_start(l2w_t[:], l2wp[:])
        l2b_t = sb.tile([P, 1], DT)
        nc.sync.dma_start(l2b_t[0:C, :], l2bp[:, None])

        # feature-major activation buffers [128, SLAB] bf16
        featA = sb_feat.tile([P, SLAB], BF)
        featB = sb_feat.tile([P, SLAB], BF)
        featC = sb_feat.tile([P, SLAB], BF)
        nc.sync.dma_start(featA[:], xoT[:])

        zero_t = sb.tile([P, P], BF)
        nc.vector.memset(zero_t[:], 0.0)

        # DRAM scratch
        cA = dram.tile([SLAB, F], BF)
        cB = dram.tile([SLAB, F], BF)
        tabA = dram.tile([NT, F], BF, addr_space="Shared")
        tabB = dram.tile([NT, F], BF, addr_space="Shared")
        tabC = dram.tile([NT, F], BF, addr_space="Shared")
        pc_in = dram.tile([P, 2 * G], DT)
        pc_out = dram.tile([P, 2 * G], DT, addr_space="Shared")
        nc.sync.dma_start(cA[NPC:SLAB, :], zero_t[0:SLAB - NPC, :])
        nc.sync.dma_start(cB[NPC:SLAB, :], zero_t[0:SLAB - NPC, :])

        def conv(tab, in_feat, out_feat, Wl, Wr, bcol, contrib):
            """One SAGE conv: out_feat[:, n] = relu(mean@Wl + in@Wr + b)."""
            if not hasattr(conv, "qctr"):
                conv.qctr = 0
            for gi in groups:
                ws, colsL, cols, col0 = gi["ws"], gi["colsL"], gi["cols"], gi["col0"]
                g_t = sb_g.tile([P, GMAX, P], BF, name="g_t")
                ix = sb_idx.tile([P, GMAX * 8], dt.int16, name="ix")
                nc.sync.dma_start(ix[:, 0:cols * 8],
                                  idxp[:, col0 * 8:(col0 + cols) * 8])
                nlo = colsL * P
                nhi = (cols - colsL) * P
                nc.gpsimd.dma_gather(
                    g_t[:, 0:colsL, :], tab[0:LO], ix[:, 0:nlo // 16],
                    nlo, nlo, P, single_packet=True,
                    queue_num=conv.qctr % 4)
                conv.qctr += 1
                nc.gpsimd.dma_gather(
                    g_t[:, colsL:cols, :], tab[LO:NT], ix[:, nlo // 16:cols * 8],
                    nhi, nhi, P, single_packet=True,
                    queue_num=conv.qctr % 4)
                conv.qctr += 1

                for w in ws:
                    agg = ps_agg.tile([P, P], dt.float32, name="agg")
                    # tile columns of this window inside g_t
                    lo0 = (slot_base[w] // P) - col0
                    hi0 = (hslot_base[w] // P) - col0
                    jcols = ([lo0 + k for k in range(int(gL[w]))] +
                             [hi0 + k for k in range(int(gH[w]))])
                    njc = len(jcols)
                    for ji, j in enumerate(jcols):
                        it = sb_ind.tile([P, P], BF, name="it")
                        r0 = (col0 + j) * P
                        nc.sync.dma_start(it[:], indp[r0:r0 + P, :])
                        nc.tensor.matmul(agg[:], g_t[:, j, :], it[:],
                                         start=(ji == 0), stop=(ji == njc - 1))
                    mean_sb = sb_ms.tile([P, P], BF, name="mean_sb")
                    nc.scalar.copy(mean_sb[:], agg[:])
                    h_ps = ps_mm.tile([P, P], dt.float32, name="h_ps", tag="mm")
                    nc.tensor.matmul(h_ps[:], Wl[:], mean_sb[:], start=True, stop=False)
                    nc.tensor.matmul(h_ps[:], Wr[:], in_feat[:, w * P:(w + 1) * P],
                                     start=False, stop=True)
                    nc.scalar.activation(out_feat[:, w * P:(w + 1) * P], h_ps[:],
                                         Act.Relu, bias=bcol[:], scale=1.0)
                    if contrib is not None:
                        rows = min(P, NPC - w * P)
                        hnm_ps = ps_tr.tile([P, P], BF, name="hnm_ps", tag="tr")
                        nc.tensor.transpose(hnm_ps[:], out_feat[:, w * P:(w + 1) * P], id_t[:])
                        hnm_sb = sb_ms.tile([P, P], BF, name="hnm_sb")
                        nc.scalar.copy(hnm_sb[:], hnm_ps[:])
                        nc.scalar.dma_start(contrib[w * P:w * P + rows, :], hnm_sb[0:rows, :])

        def jk(h1, h2, hout, Wlin, bcol, contrib, pool_sb):
            pool_ps = ps_pool.tile([P, G], dt.float32, name="pool_ps")
            for w in range(NW):
                h_ps = ps_mm.tile([P, P], dt.float32, name="jk_ps", tag="mm")
                nc.tensor.matmul(h_ps[:], Wlin[:, 0, :], h1[:, w * P:(w + 1) * P], start=True, stop=False)
                nc.tensor.matmul(h_ps[:], Wlin[:, 1, :], h2[:, w * P:(w + 1) * P], start=False, stop=True)
                nc.scalar.activation(hout[:, w * P:(w + 1) * P], h_ps[:],
                                     Act.Relu, bias=bcol[:], scale=1.0)
                hnm_ps = ps_tr.tile([P, P], BF, name="jknm_ps", tag="tr")
                nc.tensor.transpose(hnm_ps[:], hout[:, w * P:(w + 1) * P], id_t[:])
                hnm_sb = sb_ms.tile([P, P], BF, name="jknm_sb")
                nc.scalar.copy(hnm_sb[:], hnm_ps[:])
                if contrib is not None:
                    rows = min(P, NPC - w * P)
                    nc.scalar.dma_start(contrib[w * P:w * P + rows, :], hnm_sb[0:rows, :])
                pind = sb_pi.tile([P, G], BF, name="pind")
                nc.sync.dma_start(pind[:], poolp[w])
                nc.tensor.matmul(pool_ps[:], hnm_sb[:], pind[:],
                                 start=(w == 0), stop=(w == NW - 1))
            nc.vector.tensor_copy(pool_sb[:], pool_ps[:])

        def allgather(contrib, tab):
            nc.gpsimd.collective_compute(
                "AllGather", Alu.bypass, ins=[contrib[:]], outs=[tab[:]],
                replica_groups=[list(range(NCORES))])

        # ---------------- block 0
        conv(xt, featA, featB, wt["b0_Wl1"], wt["b0_Wr1"], wt["b0_b1"], cA)   # h1
        allgather(cA, tabA)
        if n_convs >= 2:
            conv(tabA, featB, featC, wt["b0_Wl2"], wt["b0_Wr2"], wt["b0_b2"], None)  # h2
            p0_sb = sb.tile([P, G], DT)
            jk(featB, featC, featA, wt["b0_Wlin"], wt["b0_blin"], cB, p0_sb)  # h -> featA
            allgather(cB, tabB)
        if n_convs >= 3:
            conv(tabB, featA, featB, wt["b1_Wl1"], wt["b1_Wr1"], wt["b1_b1"], cA)  # h1'
            allgather(cA, tabC)
        if n_convs >= 4:
            conv(tabC, featB, featC, wt["b1_Wl2"], wt["b1_Wr2"], wt["b1_b2"], None)  # h2'
            p1_sb = sb.tile([P, G], DT)
            jk(featB, featC, featA, wt["b1_Wlin"], wt["b1_blin"], None, p1_sb)

            # ---------------- pooling allreduce + head
            nc.sync.dma_start(pc_in[:, 0:G], p0_sb[:])
            nc.sync.dma_start(pc_in[:, G:2 * G], p1_sb[:])
            nc.gpsimd.collective_compute(
                "AllReduce", Alu.add, ins=[pc_in[:]], outs=[pc_out[:]],
                replica_groups=[list(range(NCORES))])
            pools_sb = sb.tile([P, 2 * G], DT)
            nc.sync.dma_start(pools_sb[:], pc_out[:])

            # BN (folded) per feature chunk -> bf16 for the head matmuls
            gbn = sb.tile([P, 2, G], BF)
            for k in range(2):
                nc.vector.tensor_scalar(gbn[:, k, :], pools_sb[:, k * G:(k + 1) * G],
                                        bns_t[:, k:k + 1], bnt_t[:, k:k + 1],
                                        Alu.mult, Alu.add)
            l1_ps = ps_mm.tile([P, G], dt.float32, name="l1_ps", tag="mm")
            for k in range(2):
                nc.tensor.matmul(l1_ps[:], l1w_t[:, k, :], gbn[:, k, :],
                                 start=(k == 0), stop=(k == 1))
            z1 = sb.tile([P, G], BF)
            nc.vector.tensor_scalar(z1[:], l1_ps[:], l1b_t[:], 0.0, Alu.add, Alu.max)
            l2_ps = ps_mm.tile([P, G], dt.float32, name="l2_ps", tag="mm")
            nc.tensor.matmul(l2_ps[0:C, :], l2w_t[:], z1[:], start=True, stop=True)
            z2 = sb.tile([P, G], DT)
            nc.vector.tensor_scalar(z2[0:C, :], l2_ps[0:C, :], l2b_t[0:C, :], None, Alu.add)

            # softmax over C (partition dim) -> transpose to [G, C] first
            zbf = sb.tile([P, G], BF)
            nc.vector.tensor_copy(zbf[0:C, :], z2[0:C, :])
            for half in range(2):
                zt_ps = ps_mm.tile([P, C], BF, name="zt_ps", tag="mm")
                nc.tensor.transpose(zt_ps[:, 0:C], zbf[0:C, half * P:(half + 1) * P], id_t[0:C, 0:C])
                znm = sb.tile([P, C], DT, name=f"znm{half}")
                nc.vector.tensor_copy(znm[:], zt_ps[:, 0:C])
                nmax = sb.tile([P, 1], DT, name=f"nmax{half}")
                nc.vector.tensor_reduce(nmax[:], znm[:], mybir.AxisListType.X, Alu.max, negate=True)
                e_t = sb.tile([P, C], DT, name=f"e_t{half}")
                nc.scalar.activation(e_t[:], znm[:], Act.Exp,
                                     bias=nmax[:], scale=1.0)
                ssum = sb.tile([P, 1], DT, name=f"ssum{half}")
                nc.vector.tensor_reduce(ssum[:], e_t[:], mybir.AxisListType.X, Alu.add)
                rcp = sb.tile([P, 1], DT, name=f"rcp{half}")
                nc.vector.reciprocal(rcp[:], ssum[:])
                sm = sb.tile([P, C], DT, name=f"sm{half}")
                nc.vector.tensor_scalar(sm[:], e_t[:], rcp[:], None, Alu.mult)
                nc.sync.dma_start(out[half * P:(half + 1) * P, :], sm[:])

        if debug_tables:
            for tabsrc, dbg in ((tabA, dbgA), (tabB, dbgB)):
                for r in range(NT // P):
                    st = sb_ms.tile([P, F], BF, name="dbg_st")
                    nc.sync.dma_start(st[:], tabsrc[r * P:(r + 1) * P, :])
                    st2 = sb_ms.tile([P, F], DT, name="dbg_st2")
                    nc.vector.tensor_copy(st2[:], st[:])
                    nc.sync.dma_start(dbg[r * P:(r + 1) * P, :], st2[:])

    nc.compile()
    return nc


# ------------------------------------------------------------------ runtime
def _install_profile_hook():
    try:
        from trn_agent_boot.trn_boot import _ntff_profile_via_ctypes
        hook = _ntff_profile_via_ctypes("/opt/axon/libaxon_pjrt.so")
        m = types.ModuleType("antenv.axon_hooks")
        m.get_axon_ntff_profile_hook = lambda: hook
        sys.modules.setdefault("antenv.axon_hooks", m)
    except Exception:
        pass


def kernel(**inputs):
    from concourse.bass_utils import run_bass_kernel_spmd

    n_convs = int(os.environ.get("KNC_CONVS", "4"))
    debug_tables = bool(int(os.environ.get("KDBG", "0")))
    trace = bool(int(os.environ.get("KTRACE", "0")))
    if trace:
        _install_profile_hook()

    sched, in_maps = _host_inputs(inputs)

    key = (n_convs, debug_tables, int(sched["tot_cols"]), int(sched["gmax"]))
    nc = _prog_cache.get(key)
    if nc is None:
        nc = _build_program(sched, n_convs=n_convs, debug_tables=debug_tables)
        _prog_cache[key] = nc

    res = run_bass_kernel_spmd(nc, in_maps, list(range(NCORES)), trace=trace)
    kernel.last_result = res
    out = res.results[0]["out"].astype(np.float32)
    return out


# revision 6
# speedup vs baseline: 2.1585x; 2.1585x over previous
"""GraphSAGE 2-block GNN (nn_BaselineModel_80607946211554) on 8 TRN2 NeuronCores.

Strategy: destination-node sharding, bf16 datapath. Each core owns 6250
contiguous nodes. Node-feature tables are replicated per-core in DRAM in a
slab layout (node n -> row (n//6250)*6272 + n%6250, 22 zero pad rows/slab).
Neighbor mean-aggregation per 128-dst window: dma_gather of bf16 source rows
(edges sorted by dst, host-preprocessed, sections padded to 128 only), then
for each 128-slot tile a DVE-generated indicator (iota==dloc)*inv_deg feeds a
PE matmul accumulating mean^T directly in PSUM. SAGE linears run
feature-major (weights stationary); PSUM->SBUF copies and bias+ReLU run on
the Activation engine. Intermediate tables rebuilt via bf16 AllGather; graph
pooling is a one-hot matmul; the MLP head + softmax is replicated per core.

Self-contained: hardcodes all shapes for the fixed problem instance.
"""
import os
import sys
import types
import numpy as np

N = 50000
E = 1600000
G = 256
F = 128
HID = 128
C = 10
NCORES = 8
NPC = N // NCORES            # 6250 nodes per core
SLAB = 6272                  # slab rows (6250 + 22 zero pad)
NT = NCORES * SLAB           # 50176 table rows
LO = 4 * SLAB                # 25088; table rows < LO hold nodes < 25000
PADROW = 6250                # zero row (local index in both lo/hi views)
P = 128
NW = (NPC + P - 1) // P      # 49 dst windows per core
EPS = 1e-5
GCAP = 112                   # max 128-slot tiles per gather group (3.7MB bf16)

_prog_cache = {}


def _bf16(a):
    import concourse.mybir as mybir
    return np.asarray(a, np.float32).astype(mybir.dt.np(mybir.dt.bfloat16))


def _wrap16(vals):
    """int64 slot values (len mult of 16) -> [128, n/16] int16 wrapped."""
    n = len(vals)
    arr = vals.reshape(n // 16, 16).T.astype(np.int16)   # [16, n/16]
    return np.tile(arr, (8, 1))                           # [128, n/16]


def _wrap128(vals):
    """[S] -> [128, S/128]: slot s -> [s%128, s//128]."""
    return vals.reshape(-1, 128).T.copy()


def _build_schedule(src, dst, invd_full):
    """Static shared schedule + per-core gather index / metadata arrays."""
    core_edges = []
    CL = np.zeros((NCORES, NW), np.int64)
    CH = np.zeros((NCORES, NW), np.int64)
    for c in range(NCORES):
        m = (dst >= c * NPC) & (dst < (c + 1) * NPC)
        s = src[m].astype(np.int64)
        d = (dst[m] - c * NPC).astype(np.int64)
        hi = (s >= N // 2).astype(np.int64)
        w = d >> 7
        order = np.lexsort((d, hi, w))
        s, d, hi, w = s[order], d[order], hi[order], w[order]
        core_edges.append((s, d, hi, w))
        cnt = np.bincount(w * 2 + hi, minlength=NW * 2).reshape(NW, 2)
        CL[c], CH[c] = cnt[:, 0], cnt[:, 1]

    nL = np.maximum(((CL.max(0) + 127) // P) * P, P)
    nH = np.maximum(((CH.max(0) + 127) // P) * P, P)
    gL, gH = nL // P, nH // P                     # tiles per section
    ngrp = gL + gH

    # pack consecutive windows into gather groups of <= GCAP tiles
    groups = []
    cur = []
    cur_cols = 0
    for w in range(NW):
        if cur and cur_cols + ngrp[w] > GCAP:
            groups.append(cur)
            cur, cur_cols = [], 0
        cur.append(w)
        cur_cols += int(ngrp[w])
    if cur:
        groups.append(cur)

    # layouts
    ginfo = []       # per group: dict
    slot_base = np.zeros(NW, np.int64)   # base slot of lo section of window
    hslot_base = np.zeros(NW, np.int64)  # base slot of hi section of window
    col0 = 0         # running tile column over all groups
    for ws in groups:
        colsL = int(gL[ws].sum())
        cols = int(ngrp[ws].sum())
        off = 0
        for w in ws:
            slot_base[w] = (col0 + off) * P
            off += int(gL[w])
        for w in ws:
            hslot_base[w] = (col0 + off) * P
            off += int(gH[w])
        ginfo.append(dict(ws=ws, colsL=colsL, cols=cols, col0=col0))
        col0 += cols
    tot_cols = col0
    S_tot = tot_cols * P

    sched = dict(nL=nL, nH=nH, gL=gL, gH=gH, ngrp=ngrp, groups=ginfo,
                 slot_base=slot_base, hslot_base=hslot_base,
                 tot_cols=tot_cols, S_tot=S_tot,
                 gmax=max(g["cols"] for g in ginfo))

    per_core = []
    for c in range(NCORES):
        s, d, hi, w = core_edges[c]
        # rank within (w, hi) section
        key = w * 2 + hi
        if len(key):
            grp_change = np.r_[True, key[1:] != key[:-1]]
            first_pos = np.flatnonzero(grp_change)
            gidx = np.cumsum(grp_change) - 1
            rank = np.arange(len(d)) - first_pos[gidx]
        else:
            rank = np.zeros(0, np.int64)
        base = np.where(hi == 1, hslot_base[w], slot_base[w])
        pos = base + rank

        trow = (s // NPC) * SLAB + s % NPC
        tval = np.where(hi == 1, trow - LO, trow)

        idx_vals = np.full(S_tot, PADROW, np.int64)
        idx_vals[pos] = tval
        ind = np.zeros((S_tot, P), np.float32)
        ind[pos, d & 127] = invd_full[c * NPC + d]
        ind_w = ind.reshape(S_tot // P, P, P).transpose(1, 0, 2).copy()

        per_core.append(dict(
            idx=_wrap16(idx_vals),
            ind=_bf16(ind_w),
        ))
    return sched, per_core


def _host_inputs(inputs):
    import concourse.mybir as mybir
    bfnp = mybir.dt.np(mybir.dt.bfloat16)
    f32 = lambda a: np.asarray(a, np.float32)
    x = f32(inputs["x"])
    ei = np.asarray(inputs["edge_index"], np.int64)
    batch = np.asarray(inputs["batch"], np.int64)
    src, dst = ei[0], ei[1]

    deg = np.bincount(dst, minlength=N).astype(np.float32)
    invd_full = (1.0 / np.maximum(deg, 1.0)).astype(np.float32)

    sched, per_core = _build_schedule(src, dst, invd_full)

    xt = np.zeros((NT, F), bfnp)
    xb = _bf16(x)
    for r in range(NCORES):
        xt[r * SLAB:r * SLAB + NPC] = xb[r * NPC:(r + 1) * NPC]

    ident = np.eye(P, dtype=np.float32)

    # BN folding
    s_bn = f32(inputs["bn_gamma"]) / np.sqrt(f32(inputs["bn_rv"]) + EPS)
    t_bn = f32(inputs["bn_beta"]) - f32(inputs["bn_rm"]) * s_bn
    bns2 = s_bn.reshape(2, P).T.copy()     # [128, 2]
    bnt2 = t_bn.reshape(2, P).T.copy()

    shared = {
        "xt": xt, "ident": _bf16(ident),
        "bns2": bns2, "bnt2": bnt2,
        "l1w": _bf16(inputs["lin1_W"]), "l1b": f32(inputs["lin1_b"]),
        "l2w": _bf16(inputs["lin2_W"]), "l2b": f32(inputs["lin2_b"]),
    }
    for b in (0, 1):
        for nm in ("Wl1", "Wr1", "Wl2", "Wr2", "Wlin"):
            shared[f"b{b}_{nm}"] = _bf16(inputs[f"b{b}_{nm}"])
        for nm in ("b1", "b2", "blin"):
            shared[f"b{b}_{nm}"] = f32(inputs[f"b{b}_{nm}"])

    in_maps = []
    for c in range(NCORES):
        xoT = np.zeros((F, SLAB), bfnp)
        xoT[:, :NPC] = xb[c * NPC:(c + 1) * NPC].T
        pool_ind = np.zeros((NW, P, G), np.float32)
        bt = batch[c * NPC:(c + 1) * NPC]
        btp = np.full(NW * P, -1, np.int64)
        btp[:NPC] = bt
        btp2 = btp.reshape(NW, P)
        for wi in range(NW):
            vm = btp2[wi] >= 0
            pool_ind[wi, np.arange(P)[vm], btp2[wi][vm]] = 1.0
        im = dict(shared)
        im.update({
            "xoT": xoT, "poolind": _bf16(pool_ind),
            "idx": per_core[c]["idx"], "ind": per_core[c]["ind"],
        })
        in_maps.append(im)
    return sched, in_maps


# ------------------------------------------------------------- bass program
def _build_program(sched, n_convs=4, debug_tables=False):
    import concourse.bass as bass
    import concourse.mybir as mybir
    import concourse.tile as tile
    from concourse import bacc
    from concourse import library_config
    from contextlib import ExitStack

    dt = mybir.dt
    DT = dt.float32
    BF = dt.bfloat16
    Alu = mybir.AluOpType
    Act = mybir.ActivationFunctionType

    nL, nH, gL, gH = (sched[k] for k in ("nL", "nH", "gL", "gH"))
    groups = sched["groups"]
    slot_base, hslot_base = sched["slot_base"], sched["hslot_base"]
    GMAX = sched["gmax"]
    TOTC = sched["tot_cols"]

    nc = bacc.Bacc("TRN2", debug=False, num_swdge_queues=4)

    # ---- parameters
    xt = nc.declare_dram_parameter("xt", [NT, F], BF, isOutput=False)
    xoT = nc.declare_dram_parameter("xoT", [F, SLAB], BF, isOutput=False)
    idxp = nc.declare_dram_parameter("idx", [P, TOTC * 8], dt.int16, isOutput=False)
    indp = nc.declare_dram_parameter("ind", [P, TOTC, P], BF, isOutput=False)
    poolp = nc.declare_dram_parameter("poolind", [NW, P, G], BF, isOutput=False)
    identp = nc.declare_dram_parameter("ident", [P, P], BF, isOutput=False)
    wp = {}
    for b in (0, 1):
        for nm, shp, dty in (("Wl1", [F, HID], BF), ("Wr1", [F, HID], BF),
                             ("b1", [HID], DT),
                             ("Wl2", [HID, HID], BF), ("Wr2", [HID, HID], BF),
                             ("b2", [HID], DT),
                             ("Wlin", [2 * HID, HID], BF), ("blin", [HID], DT)):
            wp[f"b{b}_{nm}"] = nc.declare_dram_parameter(f"b{b}_{nm}", shp, dty, isOutput=False)
    bns2p = nc.declare_dram_parameter("bns2", [P, 2], DT, isOutput=False)
    bnt2p = nc.declare_dram_parameter("bnt2", [P, 2], DT, isOutput=False)
    l1wp = nc.declare_dram_parameter("l1w", [2 * HID, HID], BF, isOutput=False)
    l1bp = nc.declare_dram_parameter("l1b", [HID], DT, isOutput=False)
    l2wp = nc.declare_dram_parameter("l2w", [HID, C], BF, isOutput=False)
    l2bp = nc.declare_dram_parameter("l2b", [C], DT, isOutput=False)

    out = nc.declare_dram_parameter("out", [G, C], DT, isOutput=True)
    if debug_tables:
        dbgA = nc.declare_dram_parameter("dbgA", [NT, F], DT, isOutput=True)
        dbgB = nc.declare_dram_parameter("dbgB", [NT, F], DT, isOutput=True)

    with tile.TileContext(nc) as tc, ExitStack() as ctx:
        sb = ctx.enter_context(tc.tile_pool(name="sb", bufs=1))
        sb_feat = ctx.enter_context(tc.tile_pool(name="sb_feat", bufs=1))
        sb_g = ctx.enter_context(tc.tile_pool(name="sb_g", bufs=2))
        sb_idx = ctx.enter_context(tc.tile_pool(name="sb_idx", bufs=2))
        sb_ind = ctx.enter_context(tc.tile_pool(name="sb_ind", bufs=2))
        sb_ms = ctx.enter_context(tc.tile_pool(name="sb_ms", bufs=4))
        sb_pi = ctx.enter_context(tc.tile_pool(name="sb_pi", bufs=3))
        ps_agg = ctx.enter_context(tc.tile_pool(name="ps_agg", bufs=2, space="PSUM"))
        ps_mm = ctx.enter_context(tc.tile_pool(name="ps_mm", bufs=2, space="PSUM"))
        ps_tr = ctx.enter_context(tc.tile_pool(name="ps_tr", bufs=2, space="PSUM"))
        ps_pool = ctx.enter_context(tc.tile_pool(name="ps_pool", bufs=1, space="PSUM"))
        dram = ctx.enter_context(tc.tile_pool(name="dram", bufs=1, space="DRAM"))

        nc.gpsimd.load_library(library_config.mlp)

        # ---- constants into SBUF
        id_t = sb.tile([P, P], BF)
        nc.sync.dma_start(id_t[:], identp[:])
        wt = {}
        for b in (0, 1):
            for nm in ("Wl1", "Wr1", "Wl2", "Wr2"):
                w_t = sb.tile([P, P], BF, name=f"w{b}{nm}")
                nc.sync.dma_start(w_t[:], wp[f"b{b}_{nm}"][:])
                wt[f"b{b}_{nm}"] = w_t
            wlin_t = sb.tile([P, 2, P], BF, name=f"w{b}lin")
            nc.sync.dma_start(wlin_t[:, 0, :], wp[f"b{b}_Wlin"][0:P, :])
            nc.sync.dma_start(wlin_t[:, 1, :], wp[f"b{b}_Wlin"][P:2 * P, :])
            wt[f"b{b}_Wlin"] = wlin_t
            for nm in ("b1", "b2", "blin"):
                b_t = sb.tile([P, 1], DT, name=f"b{b}{nm}")
                nc.sync.dma_start(b_t[:], wp[f"b{b}_{nm}"][:, None])
                wt[f"b{b}_{nm}"] = b_t
        bns_t = sb.tile([P, 2], DT)
        nc.sync.dma_start(bns_t[:], bns2p[:])
        bnt_t = sb.tile([P, 2], DT)
        nc.sync.dma_start(bnt_t[:], bnt2p[:])
        l1w_t = sb.tile([P, 2, P], BF)
        nc.sync.dma_start(l1w_t[:, 0, :], l1wp[0:P, :])
        nc.sync.dma_start(l1w_t[:, 1, :], l1wp[P:2 * P, :])
        l1b_t = sb.tile([P, 1], DT)
        nc.sync.dma_start(l1b_t[:], l1bp[:, None])
        l2w_t = sb.tile([P, C], BF)
        nc.sync.dma_start(l2w_t[:], l2wp[:])
        l2b_t = sb.tile([P, 1], DT)
        nc.sync.dma_start(l2b_t[0:C, :], l2bp[:, None])

        # feature-major activation buffers [128, SLAB] bf16
        featA = sb_feat.tile([P, SLAB], BF)
        featB = sb_feat.tile([P, SLAB], BF)
        featC = sb_feat.tile([P, SLAB], BF)
        nc.sync.dma_start(featA[:], xoT[:])

        zero_t = sb.tile([P, P], BF)
        nc.vector.memset(zero_t[:], 0.0)

        # DRAM scratch
        cA = dram.tile([SLAB, F], BF)
        cB = dram.tile([SLAB, F], BF)
        tabA = dram.tile([NT, F], BF, addr_space="Shared")
        tabB = dram.tile([NT, F], BF, addr_space="Shared")
        tabC = dram.tile([NT, F], BF, addr_space="Shared")
        pc_in = dram.tile([P, 2 * G], DT)
        pc_out = dram.tile([P, 2 * G], DT, addr_space="Shared")
        nc.sync.dma_start(cA[NPC:SLAB, :], zero_t[0:SLAB - NPC, :])
        nc.sync.dma_start(cB[NPC:SLAB, :], zero_t[0:SLAB - NPC, :])

        def conv(tab, in_feat, out_feat, Wl, Wr, bcol, contrib):
            """One SAGE conv: out_feat[:, n] = relu(mean@Wl + in@Wr + b)."""
            if not hasattr(conv, "qctr"):
                conv.qctr = 0
            for gi in groups:
                ws, colsL, cols, col0 = gi["ws"], gi["colsL"], gi["cols"], gi["col0"]
                g_t = sb_g.tile([P, GMAX, P], BF, name="g_t")
                ix = sb_idx.tile([P, GMAX * 8], dt.int16, name="ix")
                it3 = sb_ind.tile([P, GMAX, P], BF, name="it3")
                nc.sync.dma_start(ix[:, 0:cols * 8],
                                  idxp[:, col0 * 8:(col0 + cols) * 8])
                nc.sync.dma_start(it3[:, 0:cols, :], indp[:, col0:col0 + cols, :])
                nlo = colsL * P
                nhi = (cols - colsL) * P
                nc.gpsimd.dma_gather(
                    g_t[:, 0:colsL, :], tab[0:LO], ix[:, 0:nlo // 16],
                    nlo, nlo, P, single_packet=True,
                    queue_num=conv.qctr % 4)
                conv.qctr += 1
                nc.gpsimd.dma_gather(
                    g_t[:, colsL:cols, :], tab[LO:NT], ix[:, nlo // 16:cols * 8],
                    nhi, nhi, P, single_packet=True,
                    queue_num=conv.qctr % 4)
                conv.qctr += 1

                for w in ws:
                    agg = ps_agg.tile([P, P], dt.float32, name="agg")
                    # tile columns of this window inside g_t
                    lo0 = (slot_base[w] // P) - col0
                    hi0 = (hslot_base[w] // P) - col0
                    jcols = ([lo0 + k for k in range(int(gL[w]))] +
                             [hi0 + k for k in range(int(gH[w]))])
                    njc = len(jcols)
                    for ji, j in enumerate(jcols):
                        nc.tensor.matmul(agg[:], g_t[:, j, :], it3[:, j, :],
                                         start=(ji == 0), stop=(ji == njc - 1))
                    mean_sb = sb_ms.tile([P, P], BF, name="mean_sb")
                    nc.scalar.copy(mean_sb[:], agg[:])
                    h_ps = ps_mm.tile([P, P], dt.float32, name="h_ps", tag="mm")
                    nc.tensor.matmul(h_ps[:], Wl[:], mean_sb[:], start=True, stop=False)
                    nc.tensor.matmul(h_ps[:], Wr[:], in_feat[:, w * P:(w + 1) * P],
                                     start=False, stop=True)
                    nc.scalar.activation(out_feat[:, w * P:(w + 1) * P], h_ps[:],
                                         Act.Relu, bias=bcol[:], scale=1.0)
                    if contrib is not None:
                        rows = min(P, NPC - w * P)
                        hnm_ps = ps_tr.tile([P, P], BF, name="hnm_ps", tag="tr")
                        nc.tensor.transpose(hnm_ps[:], out_feat[:, w * P:(w + 1) * P], id_t[:])
                        hnm_sb = sb_ms.tile([P, P], BF, name="hnm_sb")
                        nc.scalar.copy(hnm_sb[:], hnm_ps[:])
                        nc.scalar.dma_start(contrib[w * P:w * P + rows, :], hnm_sb[0:rows, :])

        def jk(h1, h2, hout, Wlin, bcol, contrib, pool_sb):
            pool_ps = ps_pool.tile([P, G], dt.float32, name="pool_ps")
            for w in range(NW):
                h_ps = ps_mm.tile([P, P], dt.float32, name="jk_ps", tag="mm")
                nc.tensor.matmul(h_ps[:], Wlin[:, 0, :], h1[:, w * P:(w + 1) * P], start=True, stop=False)
                nc.tensor.matmul(h_ps[:], Wlin[:, 1, :], h2[:, w * P:(w + 1) * P], start=False, stop=True)
                nc.scalar.activation(hout[:, w * P:(w + 1) * P], h_ps[:],
                                     Act.Relu, bias=bcol[:], scale=1.0)
                hnm_ps = ps_tr.tile([P, P], BF, name="jknm_ps", tag="tr")
                nc.tensor.transpose(hnm_ps[:], hout[:, w * P:(w + 1) * P], id_t[:])
                hnm_sb = sb_ms.tile([P, P], BF, name="jknm_sb")
                nc.scalar.copy(hnm_sb[:], hnm_ps[:])
                if contrib is not None:
                    rows = min(P, NPC - w * P)
                    nc.scalar.dma_start(contrib[w * P:w * P + rows, :], hnm_sb[0:rows, :])
                pind = sb_pi.tile([P, G], BF, name="pind")
                nc.sync.dma_start(pind[:], poolp[w])
                nc.tensor.matmul(pool_ps[:], hnm_sb[:], pind[:],
                                 start=(w == 0), stop=(w == NW - 1))
            nc.vector.tensor_copy(pool_sb[:], pool_ps[:])

        def allgather(contrib, tab):
            nc.gpsimd.collective_compute(
                "AllGather", Alu.bypass, ins=[contrib[:]], outs=[tab[:]],
                replica_groups=[list(range(NCORES))])

        # ---------------- block 0
        conv(xt, featA, featB, wt["b0_Wl1"], wt["b0_Wr1"], wt["b0_b1"], cA)   # h1
        allgather(cA, tabA)
        if n_convs >= 2:
            conv(tabA, featB, featC, wt["b0_Wl2"], wt["b0_Wr2"], wt["b0_b2"], None)  # h2
            p0_sb = sb.tile([P, G], DT)
            jk(featB, featC, featA, wt["b0_Wlin"], wt["b0_blin"], cB, p0_sb)  # h -> featA
            allgather(cB, tabB)
        if n_convs >= 3:
            conv(tabB, featA, featB, wt["b1_Wl1"], wt["b1_Wr1"], wt["b1_b1"], cA)  # h1'
            allgather(cA, tabC)
        if n_convs >= 4:
            conv(tabC, featB, featC, wt["b1_Wl2"], wt["b1_Wr2"], wt["b1_b2"], None)  # h2'
            p1_sb = sb.tile([P, G], DT)
            jk(featB, featC, featA, wt["b1_Wlin"], wt["b1_blin"], None, p1_sb)

            # ---------------- pooling allreduce + head
            nc.sync.dma_start(pc_in[:, 0:G], p0_sb[:])
            nc.sync.dma_start(pc_in[:, G:2 * G], p1_sb[:])
            nc.gpsimd.collective_compute(
                "AllReduce", Alu.add, ins=[pc_in[:]], outs=[pc_out[:]],
                replica_groups=[list(range(NCORES))])
            pools_sb = sb.tile([P, 2 * G], DT)
            nc.sync.dma_start(pools_sb[:], pc_out[:])

            # BN (folded) per feature chunk -> bf16 for the head matmuls
            gbn = sb.tile([P, 2, G], BF)
            for k in range(2):
                nc.vector.tensor_scalar(gbn[:, k, :], pools_sb[:, k * G:(k + 1) * G],
                                        bns_t[:, k:k + 1], bnt_t[:, k:k + 1],
                                        Alu.mult, Alu.add)
            l1_ps = ps_mm.tile([P, G], dt.float32, name="l1_ps", tag="mm")
            for k in range(2):
                nc.tensor.matmul(l1_ps[:], l1w_t[:, k, :], gbn[:, k, :],
                                 start=(k == 0), stop=(k == 1))
            z1 = sb.tile([P, G], BF)
            nc.vector.tensor_scalar(z1[:], l1_ps[:], l1b_t[:], 0.0, Alu.add, Alu.max)
            l2_ps = ps_mm.tile([P, G], dt.float32, name="l2_ps", tag="mm")
            nc.tensor.matmul(l2_ps[0:C, :], l2w_t[:], z1[:], start=True, stop=True)
            z2 = sb.tile([P, G], DT)
            nc.vector.tensor_scalar(z2[0:C, :], l2_ps[0:C, :], l2b_t[0:C, :], None, Alu.add)

            # softmax over C (partition dim) -> transpose to [G, C] first
            zbf = sb.tile([P, G], BF)
            nc.vector.tensor_copy(zbf[0:C, :], z2[0:C, :])
            for half in range(2):
                zt_ps = ps_mm.tile([P, C], BF, name="zt_ps", tag="mm")
                nc.tensor.transpose(zt_ps[:, 0:C], zbf[0:C, half * P:(half + 1) * P], id_t[0:C, 0:C])
                znm = sb.tile([P, C], DT, name=f"znm{half}")
                nc.vector.tensor_copy(znm[:], zt_ps[:, 0:C])
                nmax = sb.tile([P, 1], DT, name=f"nmax{half}")
                nc.vector.tensor_reduce(nmax[:], znm[:], mybir.AxisListType.X, Alu.max, negate=True)
                e_t = sb.tile([P, C], DT, name=f"e_t{half}")
                nc.scalar.activation(e_t[:], znm[:], Act.Exp,
                                     bias=nmax[:], scale=1.0)
                ssum = sb.tile([P, 1], DT, name=f"ssum{half}")
                nc.vector.tensor_reduce(ssum[:], e_t[:], mybir.AxisListType.X, Alu.add)
                rcp = sb.tile([P, 1], DT, name=f"rcp{half}")
                nc.vector.reciprocal(rcp[:], ssum[:])
                sm = sb.tile([P, C], DT, name=f"sm{half}")
                nc.vector.tensor_scalar(sm[:], e_t[:], rcp[:], None, Alu.mult)
                nc.sync.dma_start(out[half * P:(half + 1) * P, :], sm[:])

        if debug_tables:
            for tabsrc, dbg in ((tabA, dbgA), (tabB, dbgB)):
                for r in range(NT // P):
                    st = sb_ms.tile([P, F], BF, name="dbg_st")
                    nc.sync.dma_start(st[:], tabsrc[r * P:(r + 1) * P, :])
                    st2 = sb_ms.tile([P, F], DT, name="dbg_st2")
                    nc.vector.tensor_copy(st2[:], st[:])
                    nc.sync.dma_start(dbg[r * P:(r + 1) * P, :], st2[:])

    nc.compile()
    return nc


# ------------------------------------------------------------------ runtime
def _install_profile_hook():
    try:
        from trn_agent_boot.trn_boot import _ntff_profile_via_ctypes
        hook = _ntff_profile_via_ctypes("/opt/axon/libaxon_pjrt.so")
        m = types.ModuleType("antenv.axon_hooks")
        m.get_axon_ntff_profile_hook = lambda: hook
        sys.modules.setdefault("antenv.axon_hooks", m)
    except Exception:
        pass


def kernel(**inputs):
    from concourse.bass_utils import run_bass_kernel_spmd

    n_convs = int(os.environ.get("KNC_CONVS", "4"))
    debug_tables = bool(int(os.environ.get("KDBG", "0")))
    trace = bool(int(os.environ.get("KTRACE", "0")))
    if trace:
        _install_profile_hook()

    sched, in_maps = _host_inputs(inputs)

    key = (n_convs, debug_tables, int(sched["tot_cols"]), int(sched["gmax"]))
    nc = _prog_cache.get(key)
    if nc is None:
        nc = _build_program(sched, n_convs=n_convs, debug_tables=debug_tables)
        _prog_cache[key] = nc

    res = run_bass_kernel_spmd(nc, in_maps, list(range(NCORES)), trace=trace)
    kernel.last_result = res
    out = res.results[0]["out"].astype(np.float32)
    return out


# revision 7
# speedup vs baseline: 2.1764x; 1.0083x over previous
"""GraphSAGE 2-block GNN (nn_BaselineModel_80607946211554) on 8 TRN2 NeuronCores.

Strategy: destination-node sharding, bf16 datapath. Each core owns 6250
contiguous nodes. Node-feature tables are replicated per-core in DRAM in a
slab layout (node n -> row (n//6250)*6272 + n%6250, 22 zero pad rows/slab).
Neighbor mean-aggregation per 128-dst window: dma_gather of bf16 source rows
(edges sorted by dst, host-preprocessed, sections padded to 128 only), then
for each 128-slot tile a DVE-generated indicator (iota==dloc)*inv_deg feeds a
PE matmul accumulating mean^T directly in PSUM. SAGE linears run
feature-major (weights stationary); PSUM->SBUF copies and bias+ReLU run on
the Activation engine. Intermediate tables rebuilt via bf16 AllGather; graph
pooling is a one-hot matmul; the MLP head + softmax is replicated per core.

Self-contained: hardcodes all shapes for the fixed problem instance.
"""
import os
import sys
import types
import numpy as np

N = 50000
E = 1600000
G = 256
F = 128
HID = 128
C = 10
NCORES = 8
NPC = N // NCORES            # 6250 nodes per core
SLAB = 6272                  # slab rows (6250 + 22 zero pad)
NT = NCORES * SLAB           # 50176 table rows
LO = 4 * SLAB                # 25088; table rows < LO hold nodes < 25000
PADROW = 6250                # zero row (local index in both lo/hi views)
P = 128
NW = (NPC + P - 1) // P      # 49 dst windows per core
EPS = 1e-5
GCAP = 112                   # max 128-slot tiles per gather group (3.7MB bf16)

_prog_cache = {}


def _bf16(a):
    import concourse.mybir as mybir
    return np.asarray(a, np.float32).astype(mybir.dt.np(mybir.dt.bfloat16))


def _fp8(a):
    import concourse.mybir as mybir
    return np.asarray(a, np.float32).astype(mybir.dt.np(mybir.dt.float8e4))


def _wrap16(vals):
    """int64 slot values (len mult of 16) -> [128, n/16] int16 wrapped."""
    n = len(vals)
    arr = vals.reshape(n // 16, 16).T.astype(np.int16)   # [16, n/16]
    return np.tile(arr, (8, 1))                           # [128, n/16]


def _wrap128(vals):
    """[S] -> [128, S/128]: slot s -> [s%128, s//128]."""
    return vals.reshape(-1, 128).T.copy()


def _build_schedule(src, dst, invd_full):
    """Static shared schedule + per-core gather index / metadata arrays."""
    core_edges = []
    CL = np.zeros((NCORES, NW), np.int64)
    CH = np.zeros((NCORES, NW), np.int64)
    for c in range(NCORES):
        m = (dst >= c * NPC) & (dst < (c + 1) * NPC)
        s = src[m].astype(np.int64)
        d = (dst[m] - c * NPC).astype(np.int64)
        hi = (s >= N // 2).astype(np.int64)
        w = d >> 7
        order = np.lexsort((d, hi, w))
        s, d, hi, w = s[order], d[order], hi[order], w[order]
        core_edges.append((s, d, hi, w))
        cnt = np.bincount(w * 2 + hi, minlength=NW * 2).reshape(NW, 2)
        CL[c], CH[c] = cnt[:, 0], cnt[:, 1]

    nL = np.maximum(((CL.max(0) + 127) // P) * P, P)
    nH = np.maximum(((CH.max(0) + 127) // P) * P, P)
    gL, gH = nL // P, nH // P                     # tiles per section
    ngrp = gL + gH

    # pack consecutive windows into gather groups of <= GCAP tiles
    groups = []
    cur = []
    cur_cols = 0
    for w in range(NW):
        if cur and cur_cols + ngrp[w] > GCAP:
            groups.append(cur)
            cur, cur_cols = [], 0
        cur.append(w)
        cur_cols += int(ngrp[w])
    if cur:
        groups.append(cur)

    # layouts
    ginfo = []       # per group: dict
    slot_base = np.zeros(NW, np.int64)   # base slot of lo section of window
    hslot_base = np.zeros(NW, np.int64)  # base slot of hi section of window
    col0 = 0         # running tile column over all groups
    for ws in groups:
        colsL = int(gL[ws].sum())
        cols = int(ngrp[ws].sum())
        off = 0
        for w in ws:
            slot_base[w] = (col0 + off) * P
            off += int(gL[w])
        for w in ws:
            hslot_base[w] = (col0 + off) * P
            off += int(gH[w])
        ginfo.append(dict(ws=ws, colsL=colsL, cols=cols, col0=col0))
        col0 += cols
    tot_cols = col0
    S_tot = tot_cols * P

    sched = dict(nL=nL, nH=nH, gL=gL, gH=gH, ngrp=ngrp, groups=ginfo,
                 slot_base=slot_base, hslot_base=hslot_base,
                 tot_cols=tot_cols, S_tot=S_tot,
                 gmax=max(g["cols"] for g in ginfo))

    per_core = []
    for c in range(NCORES):
        s, d, hi, w = core_edges[c]
        # rank within (w, hi) section
        key = w * 2 + hi
        if len(key):
            grp_change = np.r_[True, key[1:] != key[:-1]]
            first_pos = np.flatnonzero(grp_change)
            gidx = np.cumsum(grp_change) - 1
            rank = np.arange(len(d)) - first_pos[gidx]
        else:
            rank = np.zeros(0, np.int64)
        base = np.where(hi == 1, hslot_base[w], slot_base[w])
        pos = base + rank

        trow = (s // NPC) * SLAB + s % NPC
        tval = np.where(hi == 1, trow - LO, trow)

        idx_vals = np.full(S_tot, PADROW, np.int64)
        idx_vals[pos] = tval
        ind = np.zeros((S_tot, P), np.float32)
        ind[pos, d & 127] = 1.0
        ind_w = ind.reshape(S_tot // P, P, P).transpose(1, 0, 2).copy()

        per_core.append(dict(
            idx=_wrap16(idx_vals),
            ind=_fp8(ind_w),
        ))
    return sched, per_core


def _host_inputs(inputs):
    import concourse.mybir as mybir
    bfnp = mybir.dt.np(mybir.dt.bfloat16)
    f32 = lambda a: np.asarray(a, np.float32)
    x = f32(inputs["x"])
    ei = np.asarray(inputs["edge_index"], np.int64)
    batch = np.asarray(inputs["batch"], np.int64)
    src, dst = ei[0], ei[1]

    deg = np.bincount(dst, minlength=N).astype(np.float32)
    invd_full = (1.0 / np.maximum(deg, 1.0)).astype(np.float32)

    sched, per_core = _build_schedule(src, dst, invd_full)

    xt = np.zeros((NT, F), bfnp)
    xb = _bf16(x)
    for r in range(NCORES):
        xt[r * SLAB:r * SLAB + NPC] = xb[r * NPC:(r + 1) * NPC]

    ident = np.eye(P, dtype=np.float32)

    # BN folding
    s_bn = f32(inputs["bn_gamma"]) / np.sqrt(f32(inputs["bn_rv"]) + EPS)
    t_bn = f32(inputs["bn_beta"]) - f32(inputs["bn_rm"]) * s_bn
    bns2 = s_bn.reshape(2, P).T.copy()     # [128, 2]
    bnt2 = t_bn.reshape(2, P).T.copy()

    shared = {
        "xt": xt, "ident": _bf16(ident),
        "bns2": bns2, "bnt2": bnt2,
        "l1w": _bf16(inputs["lin1_W"]), "l1b": f32(inputs["lin1_b"]),
        "l2w": _bf16(inputs["lin2_W"]), "l2b": f32(inputs["lin2_b"]),
    }
    for b in (0, 1):
        for nm in ("Wl1", "Wr1", "Wl2", "Wr2", "Wlin"):
            shared[f"b{b}_{nm}"] = _bf16(inputs[f"b{b}_{nm}"])
        for nm in ("b1", "b2", "blin"):
            shared[f"b{b}_{nm}"] = f32(inputs[f"b{b}_{nm}"])

    in_maps = []
    for c in range(NCORES):
        xoT = np.zeros((F, SLAB), bfnp)
        xoT[:, :NPC] = xb[c * NPC:(c + 1) * NPC].T
        ivb = np.zeros((P, SLAB), np.float32)
        ivb[:, :NPC] = invd_full[c * NPC:(c + 1) * NPC][None, :]
        pool_ind = np.zeros((NW, P, G), np.float32)
        bt = batch[c * NPC:(c + 1) * NPC]
        btp = np.full(NW * P, -1, np.int64)
        btp[:NPC] = bt
        btp2 = btp.reshape(NW, P)
        for wi in range(NW):
            vm = btp2[wi] >= 0
            pool_ind[wi, np.arange(P)[vm], btp2[wi][vm]] = 1.0
        im = dict(shared)
        im.update({
            "xoT": xoT, "poolind": _bf16(pool_ind), "invb": _bf16(ivb),
            "idx": per_core[c]["idx"], "ind": per_core[c]["ind"],
        })
        in_maps.append(im)
    return sched, in_maps


# ------------------------------------------------------------- bass program
def _build_program(sched, n_convs=4, debug_tables=False):
    import concourse.bass as bass
    import concourse.mybir as mybir
    import concourse.tile as tile
    from concourse import bacc
    from concourse import library_config
    from contextlib import ExitStack

    dt = mybir.dt
    DT = dt.float32
    BF = dt.bfloat16
    Alu = mybir.AluOpType
    Act = mybir.ActivationFunctionType

    nL, nH, gL, gH = (sched[k] for k in ("nL", "nH", "gL", "gH"))
    groups = sched["groups"]
    slot_base, hslot_base = sched["slot_base"], sched["hslot_base"]
    GMAX = sched["gmax"]
    TOTC = sched["tot_cols"]

    nc = bacc.Bacc("TRN2", debug=False, num_swdge_queues=4)

    # ---- parameters
    xt = nc.declare_dram_parameter("xt", [NT, F], BF, isOutput=False)
    xoT = nc.declare_dram_parameter("xoT", [F, SLAB], BF, isOutput=False)
    idxp = nc.declare_dram_parameter("idx", [P, TOTC * 8], dt.int16, isOutput=False)
    indp = nc.declare_dram_parameter("ind", [P, TOTC, P], dt.float8e4, isOutput=False)
    invbp = nc.declare_dram_parameter("invb", [P, SLAB], BF, isOutput=False)
    poolp = nc.declare_dram_parameter("poolind", [NW, P, G], BF, isOutput=False)
    identp = nc.declare_dram_parameter("ident", [P, P], BF, isOutput=False)
    wp = {}
    for b in (0, 1):
        for nm, shp, dty in (("Wl1", [F, HID], BF), ("Wr1", [F, HID], BF),
                             ("b1", [HID], DT),
                             ("Wl2", [HID, HID], BF), ("Wr2", [HID, HID], BF),
                             ("b2", [HID], DT),
                             ("Wlin", [2 * HID, HID], BF), ("blin", [HID], DT)):
            wp[f"b{b}_{nm}"] = nc.declare_dram_parameter(f"b{b}_{nm}", shp, dty, isOutput=False)
    bns2p = nc.declare_dram_parameter("bns2", [P, 2], DT, isOutput=False)
    bnt2p = nc.declare_dram_parameter("bnt2", [P, 2], DT, isOutput=False)
    l1wp = nc.declare_dram_parameter("l1w", [2 * HID, HID], BF, isOutput=False)
    l1bp = nc.declare_dram_parameter("l1b", [HID], DT, isOutput=False)
    l2wp = nc.declare_dram_parameter("l2w", [HID, C], BF, isOutput=False)
    l2bp = nc.declare_dram_parameter("l2b", [C], DT, isOutput=False)

    out = nc.declare_dram_parameter("out", [G, C], DT, isOutput=True)
    if debug_tables:
        dbgA = nc.declare_dram_parameter("dbgA", [NT, F], DT, isOutput=True)
        dbgB = nc.declare_dram_parameter("dbgB", [NT, F], DT, isOutput=True)

    with tile.TileContext(nc) as tc, ExitStack() as ctx:
        sb = ctx.enter_context(tc.tile_pool(name="sb", bufs=1))
        sb_feat = ctx.enter_context(tc.tile_pool(name="sb_feat", bufs=1))
        sb_g = ctx.enter_context(tc.tile_pool(name="sb_g", bufs=2))
        sb_idx = ctx.enter_context(tc.tile_pool(name="sb_idx", bufs=2))
        sb_ind = ctx.enter_context(tc.tile_pool(name="sb_ind", bufs=2))
        sb_ms = ctx.enter_context(tc.tile_pool(name="sb_ms", bufs=4))
        sb_pi = ctx.enter_context(tc.tile_pool(name="sb_pi", bufs=3))
        ps_agg = ctx.enter_context(tc.tile_pool(name="ps_agg", bufs=2, space="PSUM"))
        ps_mm = ctx.enter_context(tc.tile_pool(name="ps_mm", bufs=2, space="PSUM"))
        ps_tr = ctx.enter_context(tc.tile_pool(name="ps_tr", bufs=2, space="PSUM"))
        ps_pool = ctx.enter_context(tc.tile_pool(name="ps_pool", bufs=1, space="PSUM"))
        dram = ctx.enter_context(tc.tile_pool(name="dram", bufs=1, space="DRAM"))

        nc.gpsimd.load_library(library_config.mlp)

        # ---- constants into SBUF
        id_t = sb.tile([P, P], BF)
        nc.sync.dma_start(id_t[:], identp[:])
        invb_t = sb.tile([P, SLAB], BF)
        nc.sync.dma_start(invb_t[:], invbp[:])
        wt = {}
        for b in (0, 1):
            for nm in ("Wl1", "Wr1", "Wl2", "Wr2"):
                w_t = sb.tile([P, P], BF, name=f"w{b}{nm}")
                nc.sync.dma_start(w_t[:], wp[f"b{b}_{nm}"][:])
                wt[f"b{b}_{nm}"] = w_t
            wlin_t = sb.tile([P, 2, P], BF, name=f"w{b}lin")
            nc.sync.dma_start(wlin_t[:, 0, :], wp[f"b{b}_Wlin"][0:P, :])
            nc.sync.dma_start(wlin_t[:, 1, :], wp[f"b{b}_Wlin"][P:2 * P, :])
            wt[f"b{b}_Wlin"] = wlin_t
            for nm in ("b1", "b2", "blin"):
                b_t = sb.tile([P, 1], DT, name=f"b{b}{nm}")
                nc.sync.dma_start(b_t[:], wp[f"b{b}_{nm}"][:, None])
                wt[f"b{b}_{nm}"] = b_t
        bns_t = sb.tile([P, 2], DT)
        nc.sync.dma_start(bns_t[:], bns2p[:])
        bnt_t = sb.tile([P, 2], DT)
        nc.sync.dma_start(bnt_t[:], bnt2p[:])
        l1w_t = sb.tile([P, 2, P], BF)
        nc.sync.dma_start(l1w_t[:, 0, :], l1wp[0:P, :])
        nc.sync.dma_start(l1w_t[:, 1, :], l1wp[P:2 * P, :])
        l1b_t = sb.tile([P, 1], DT)
        nc.sync.dma_start(l1b_t[:], l1bp[:, None])
        l2w_t = sb.tile([P, C], BF)
        nc.sync.dma_start(l2w_t[:], l2wp[:])
        l2b_t = sb.tile([P, 1], DT)
        nc.sync.dma_start(l2b_t[0:C, :], l2bp[:, None])

        # feature-major activation buffers [128, SLAB] bf16
        featA = sb_feat.tile([P, SLAB], BF)
        featB = sb_feat.tile([P, SLAB], BF)
        featC = sb_feat.tile([P, SLAB], BF)
        nc.sync.dma_start(featA[:], xoT[:])

        zero_t = sb.tile([P, P], BF)
        nc.vector.memset(zero_t[:], 0.0)

        # DRAM scratch
        cA = dram.tile([SLAB, F], BF)
        cB = dram.tile([SLAB, F], BF)
        tabA = dram.tile([NT, F], BF, addr_space="Shared")
        tabB = dram.tile([NT, F], BF, addr_space="Shared")
        tabC = dram.tile([NT, F], BF, addr_space="Shared")
        pc_in = dram.tile([P, 2 * G], DT)
        pc_out = dram.tile([P, 2 * G], DT, addr_space="Shared")
        nc.sync.dma_start(cA[NPC:SLAB, :], zero_t[0:SLAB - NPC, :])
        nc.sync.dma_start(cB[NPC:SLAB, :], zero_t[0:SLAB - NPC, :])

        def conv(tab, in_feat, out_feat, Wl, Wr, bcol, contrib, jk=None):
            """One SAGE conv: out_feat[:, n] = relu(mean@Wl + in@Wr + b).
            If jk is given, also fuse the JumpingKnowledge linear + pooling:
            jk = (h1_feat, Wlin, blin_col, jk_contrib, hout, pool_ps)."""
            if not hasattr(conv, "qctr"):
                conv.qctr = 0
            for gi in groups:
                ws, colsL, cols, col0 = gi["ws"], gi["colsL"], gi["cols"], gi["col0"]
                g_t = sb_g.tile([P, GMAX, P], BF, name="g_t")
                ix = sb_idx.tile([P, GMAX * 8], dt.int16, name="ix")
                it3 = sb_ind.tile([P, GMAX, P], dt.float8e4, name="it3")
                nc.sync.dma_start(ix[:, 0:cols * 8],
                                  idxp[:, col0 * 8:(col0 + cols) * 8])
                nc.sync.dma_start(it3[:, 0:cols, :], indp[:, col0:col0 + cols, :])
                nlo = colsL * P
                nhi = (cols - colsL) * P
                nc.gpsimd.dma_gather(
                    g_t[:, 0:colsL, :], tab[0:LO], ix[:, 0:nlo // 16],
                    nlo, nlo, P, single_packet=True,
                    queue_num=conv.qctr % 4)
                conv.qctr += 1
                nc.gpsimd.dma_gather(
                    g_t[:, colsL:cols, :], tab[LO:NT], ix[:, nlo // 16:cols * 8],
                    nhi, nhi, P, single_packet=True,
                    queue_num=conv.qctr % 4)
                conv.qctr += 1

                for w in ws:
                    agg = ps_agg.tile([P, P], dt.float32, name="agg")
                    # tile columns of this window inside g_t
                    lo0 = (slot_base[w] // P) - col0
                    hi0 = (hslot_base[w] // P) - col0
                    jcols = ([lo0 + k for k in range(int(gL[w]))] +
                             [hi0 + k for k in range(int(gH[w]))])
                    njc = len(jcols)
                    for ji, j in enumerate(jcols):
                        nc.tensor.matmul(agg[:], g_t[:, j, :], it3[:, j, :],
                                         start=(ji == 0), stop=(ji == njc - 1))
                    mean_sb = sb_ms.tile([P, P], BF, name="mean_sb")
                    nc.vector.tensor_tensor(mean_sb[:], agg[:],
                                            invb_t[:, w * P:(w + 1) * P], Alu.mult)
                    h_ps = ps_mm.tile([P, P], dt.float32, name="h_ps", tag="mm")
                    nc.tensor.matmul(h_ps[:], Wl[:], mean_sb[:], start=True, stop=False)
                    nc.tensor.matmul(h_ps[:], Wr[:], in_feat[:, w * P:(w + 1) * P],
                                     start=False, stop=True)
                    nc.scalar.activation(out_feat[:, w * P:(w + 1) * P], h_ps[:],
                                         Act.Relu, bias=bcol[:], scale=1.0)
                    if contrib is not None:
                        rows = min(P, NPC - w * P)
                        hnm_ps = ps_tr.tile([P, P], BF, name="hnm_ps", tag="tr")
                        nc.tensor.transpose(hnm_ps[:], out_feat[:, w * P:(w + 1) * P], id_t[:])
                        hnm_sb = sb_ms.tile([P, P], BF, name="hnm_sb")
                        nc.scalar.copy(hnm_sb[:], hnm_ps[:])
                        nc.scalar.dma_start(contrib[w * P:w * P + rows, :], hnm_sb[0:rows, :])
                    if jk is not None:
                        h1f, Wlin, blc, jkcontrib, hout, pool_ps = jk
                        j_ps = ps_mm.tile([P, P], dt.float32, name="j_ps", tag="mm")
                        nc.tensor.matmul(j_ps[:], Wlin[:, 0, :], h1f[:, w * P:(w + 1) * P],
                                         start=True, stop=False)
                        nc.tensor.matmul(j_ps[:], Wlin[:, 1, :], out_feat[:, w * P:(w + 1) * P],
                                         start=False, stop=True)
                        nc.scalar.activation(hout[:, w * P:(w + 1) * P], j_ps[:],
                                             Act.Relu, bias=blc[:], scale=1.0)
                        jnm_ps = ps_tr.tile([P, P], BF, name="jnm_ps", tag="tr")
                        nc.tensor.transpose(jnm_ps[:], hout[:, w * P:(w + 1) * P], id_t[:])
                        jnm_sb = sb_ms.tile([P, P], BF, name="jnm_sb")
                        nc.scalar.copy(jnm_sb[:], jnm_ps[:])
                        if jkcontrib is not None:
                            rows = min(P, NPC - w * P)
                            nc.scalar.dma_start(jkcontrib[w * P:w * P + rows, :], jnm_sb[0:rows, :])
                        pind = sb_pi.tile([P, G], BF, name="pind")
                        nc.sync.dma_start(pind[:], poolp[w])
                        nc.tensor.matmul(pool_ps[:], jnm_sb[:], pind[:],
                                         start=(w == 0), stop=(w == NW - 1))

        def allgather(contrib, tab):
            nc.gpsimd.collective_compute(
                "AllGather", Alu.bypass, ins=[contrib[:]], outs=[tab[:]],
                replica_groups=[list(range(NCORES))])

        # ---------------- block 0
        conv(xt, featA, featB, wt["b0_Wl1"], wt["b0_Wr1"], wt["b0_b1"], cA)   # h1
        allgather(cA, tabA)
        if n_convs >= 2:
            p0_ps = ps_pool.tile([P, G], dt.float32, name="p0_ps")
            conv(tabA, featB, featC, wt["b0_Wl2"], wt["b0_Wr2"], wt["b0_b2"], None,
                 jk=(featB, wt["b0_Wlin"], wt["b0_blin"], cB, featA, p0_ps))  # h2; h -> featA
            p0_sb = sb.tile([P, G], DT)
            nc.vector.tensor_copy(p0_sb[:], p0_ps[:])
            allgather(cB, tabB)
        if n_convs >= 3:
            conv(tabB, featA, featB, wt["b1_Wl1"], wt["b1_Wr1"], wt["b1_b1"], cA)  # h1'
            allgather(cA, tabC)
        if n_convs >= 4:
            p1_ps = ps_pool.tile([P, G], dt.float32, name="p1_ps")
            conv(tabC, featB, featC, wt["b1_Wl2"], wt["b1_Wr2"], wt["b1_b2"], None,
                 jk=(featB, wt["b1_Wlin"], wt["b1_blin"], None, featA, p1_ps))  # h2'
            p1_sb = sb.tile([P, G], DT)
            nc.vector.tensor_copy(p1_sb[:], p1_ps[:])

            # ---------------- pooling allreduce + head
            nc.sync.dma_start(pc_in[:, 0:G], p0_sb[:])
            nc.sync.dma_start(pc_in[:, G:2 * G], p1_sb[:])
            nc.gpsimd.collective_compute(
                "AllReduce", Alu.add, ins=[pc_in[:]], outs=[pc_out[:]],
                replica_groups=[list(range(NCORES))])
            pools_sb = sb.tile([P, 2 * G], DT)
            nc.sync.dma_start(pools_sb[:], pc_out[:])

            # BN (folded) per feature chunk -> bf16 for the head matmuls
            gbn = sb.tile([P, 2, G], BF)
            for k in range(2):
                nc.vector.tensor_scalar(gbn[:, k, :], pools_sb[:, k * G:(k + 1) * G],
                                        bns_t[:, k:k + 1], bnt_t[:, k:k + 1],
                                        Alu.mult, Alu.add)
            l1_ps = ps_mm.tile([P, G], dt.float32, name="l1_ps", tag="mm")
            for k in range(2):
                nc.tensor.matmul(l1_ps[:], l1w_t[:, k, :], gbn[:, k, :],
                                 start=(k == 0), stop=(k == 1))
            z1 = sb.tile([P, G], BF)
            nc.vector.tensor_scalar(z1[:], l1_ps[:], l1b_t[:], 0.0, Alu.add, Alu.max)
            l2_ps = ps_mm.tile([P, G], dt.float32, name="l2_ps", tag="mm")
            nc.tensor.matmul(l2_ps[0:C, :], l2w_t[:], z1[:], start=True, stop=True)
            z2 = sb.tile([P, G], DT)
            nc.vector.tensor_scalar(z2[0:C, :], l2_ps[0:C, :], l2b_t[0:C, :], None, Alu.add)

            # softmax over C (partition dim) -> transpose to [G, C] first
            zbf = sb.tile([P, G], BF)
            nc.vector.tensor_copy(zbf[0:C, :], z2[0:C, :])
            for half in range(2):
                zt_ps = ps_mm.tile([P, C], BF, name="zt_ps", tag="mm")
                nc.tensor.transpose(zt_ps[:, 0:C], zbf[0:C, half * P:(half + 1) * P], id_t[0:C, 0:C])
                znm = sb.tile([P, C], DT, name=f"znm{half}")
                nc.vector.tensor_copy(znm[:], zt_ps[:, 0:C])
                nmax = sb.tile([P, 1], DT, name=f"nmax{half}")
                nc.vector.tensor_reduce(nmax[:], znm[:], mybir.AxisListType.X, Alu.max, negate=True)
                e_t = sb.tile([P, C], DT, name=f"e_t{half}")
                nc.scalar.activation(e_t[:], znm[:], Act.Exp,
                                     bias=nmax[:], scale=1.0)
                ssum = sb.tile([P, 1], DT, name=f"ssum{half}")
                nc.vector.tensor_reduce(ssum[:], e_t[:], mybir.AxisListType.X, Alu.add)
                rcp = sb.tile([P, 1], DT, name=f"rcp{half}")
                nc.vector.reciprocal(rcp[:], ssum[:])
                sm = sb.tile([P, C], DT, name=f"sm{half}")
                nc.vector.tensor_scalar(sm[:], e_t[:], rcp[:], None, Alu.mult)
                nc.sync.dma_start(out[half * P:(half + 1) * P, :], sm[:])

        if debug_tables:
            for tabsrc, dbg in ((tabA, dbgA), (tabB, dbgB)):
                for r in range(NT // P):
                    st = sb_ms.tile([P, F], BF, name="dbg_st")
                    nc.sync.dma_start(st[:], tabsrc[r * P:(r + 1) * P, :])
                    st2 = sb_ms.tile([P, F], DT, name="dbg_st2")
                    nc.vector.tensor_copy(st2[:], st[:])
                    nc.sync.dma_start(dbg[r * P:(r + 1) * P, :], st2[:])

    nc.compile()
    return nc


# ------------------------------------------------------------------ runtime
def _install_profile_hook():
    try:
        from trn_agent_boot.trn_boot import _ntff_profile_via_ctypes
        hook = _ntff_profile_via_ctypes("/opt/axon/libaxon_pjrt.so")
        m = types.ModuleType("antenv.axon_hooks")
        m.get_axon_ntff_profile_hook = lambda: hook
        sys.modules.setdefault("antenv.axon_hooks", m)
    except Exception:
        pass


def kernel(**inputs):
    from concourse.bass_utils import run_bass_kernel_spmd

    n_convs = int(os.environ.get("KNC_CONVS", "4"))
    debug_tables = bool(int(os.environ.get("KDBG", "0")))
    trace = bool(int(os.environ.get("KTRACE", "0")))
    if trace:
        _install_profile_hook()

    sched, in_maps = _host_inputs(inputs)

    key = (n_convs, debug_tables, int(sched["tot_cols"]), int(sched["gmax"]))
    nc = _prog_cache.get(key)
    if nc is None:
        nc = _build_program(sched, n_convs=n_convs, debug_tables=debug_tables)
        _prog_cache[key] = nc

    res = run_bass_kernel_spmd(nc, in_maps, list(range(NCORES)), trace=trace)
    kernel.last_result = res
    out = res.results[0]["out"].astype(np.float32)
    return out


# revision 8
# speedup vs baseline: 3.0518x; 1.4022x over previous
"""GraphSAGE 2-block GNN (nn_BaselineModel_80607946211554) on 8 TRN2 NeuronCores.

Strategy: destination-node sharding, bf16 datapath. Each core owns 6250
contiguous nodes. Node-feature tables are replicated per-core in DRAM in a
slab layout (node n -> row (n//6250)*6272 + n%6250, 22 zero pad rows/slab).
Neighbor mean-aggregation per 128-dst window: dma_gather of bf16 source rows
(edges sorted by dst, host-preprocessed, sections padded to 128 only), then
for each 128-slot tile a DVE-generated indicator (iota==dloc)*inv_deg feeds a
PE matmul accumulating mean^T directly in PSUM. SAGE linears run
feature-major (weights stationary); PSUM->SBUF copies and bias+ReLU run on
the Activation engine. Intermediate tables rebuilt via bf16 AllGather; graph
pooling is a one-hot matmul; the MLP head + softmax is replicated per core.

Self-contained: hardcodes all shapes for the fixed problem instance.
"""
import os
import sys
import types
import numpy as np

N = 50000
E = 1600000
G = 256
F = 128
HID = 128
C = 10
NCORES = 8
NPC = N // NCORES            # 6250 nodes per core
SLAB = 6272                  # slab rows (6250 + 22 zero pad)
NT = NCORES * SLAB           # 50176 table rows
LO = 4 * SLAB                # 25088; table rows < LO hold nodes < 25000
PADROW = 6250                # zero row (local index in both lo/hi views)
P = 128
NW = (NPC + P - 1) // P      # 49 dst windows per core
EPS = 1e-5
GCAP = 96                    # max 128-slot tiles per gather group

_prog_cache = {}


def _bf16(a):
    import concourse.mybir as mybir
    return np.asarray(a, np.float32).astype(mybir.dt.np(mybir.dt.bfloat16))


def _fp8(a):
    import concourse.mybir as mybir
    return np.asarray(a, np.float32).astype(mybir.dt.np(mybir.dt.float8e4))


def _wrap16(vals):
    """int64 slot values (len mult of 16) -> [128, n/16] int16 wrapped."""
    n = len(vals)
    arr = vals.reshape(n // 16, 16).T.astype(np.int16)   # [16, n/16]
    return np.tile(arr, (8, 1))                           # [128, n/16]


def _wrap128(vals):
    """[S] -> [128, S/128]: slot s -> [s%128, s//128]."""
    return vals.reshape(-1, 128).T.copy()


def _build_schedule(src, dst, invd_full):
    """Static shared schedule + per-core gather index / metadata arrays."""
    core_edges = []
    CL = np.zeros((NCORES, NW), np.int64)
    CH = np.zeros((NCORES, NW), np.int64)
    for c in range(NCORES):
        m = (dst >= c * NPC) & (dst < (c + 1) * NPC)
        s = src[m].astype(np.int64)
        d = (dst[m] - c * NPC).astype(np.int64)
        hi = (s >= N // 2).astype(np.int64)
        w = d >> 7
        order = np.lexsort((d, hi, w))
        s, d, hi, w = s[order], d[order], hi[order], w[order]
        core_edges.append((s, d, hi, w))
        cnt = np.bincount(w * 2 + hi, minlength=NW * 2).reshape(NW, 2)
        CL[c], CH[c] = cnt[:, 0], cnt[:, 1]

    nL = np.maximum(((CL.max(0) + 127) // P) * P, P)
    nH = np.maximum(((CH.max(0) + 127) // P) * P, P)
    gL, gH = nL // P, nH // P                     # tiles per section
    ngrp = gL + gH

    # pack consecutive windows into gather groups of <= GCAP tiles
    groups = []
    cur = []
    cur_cols = 0
    for w in range(NW):
        if cur and cur_cols + ngrp[w] > GCAP:
            groups.append(cur)
            cur, cur_cols = [], 0
        cur.append(w)
        cur_cols += int(ngrp[w])
    if cur:
        groups.append(cur)

    # layouts
    ginfo = []       # per group: dict
    slot_base = np.zeros(NW, np.int64)   # base slot of lo section of window
    hslot_base = np.zeros(NW, np.int64)  # base slot of hi section of window
    col0 = 0         # running tile column over all groups
    for ws in groups:
        colsL = int(gL[ws].sum())
        cols = int(ngrp[ws].sum())
        off = 0
        for w in ws:
            slot_base[w] = (col0 + off) * P
            off += int(gL[w])
        for w in ws:
            hslot_base[w] = (col0 + off) * P
            off += int(gH[w])
        ginfo.append(dict(ws=ws, colsL=colsL, cols=cols, col0=col0))
        col0 += cols
    tot_cols = col0
    S_tot = tot_cols * P

    sched = dict(nL=nL, nH=nH, gL=gL, gH=gH, ngrp=ngrp, groups=ginfo,
                 slot_base=slot_base, hslot_base=hslot_base,
                 tot_cols=tot_cols, S_tot=S_tot,
                 gmax=max(g["cols"] for g in ginfo))

    per_core = []
    for c in range(NCORES):
        s, d, hi, w = core_edges[c]
        # rank within (w, hi) section
        key = w * 2 + hi
        if len(key):
            grp_change = np.r_[True, key[1:] != key[:-1]]
            first_pos = np.flatnonzero(grp_change)
            gidx = np.cumsum(grp_change) - 1
            rank = np.arange(len(d)) - first_pos[gidx]
        else:
            rank = np.zeros(0, np.int64)
        base = np.where(hi == 1, hslot_base[w], slot_base[w])
        pos = base + rank

        trow = (s // NPC) * SLAB + s % NPC
        tval = np.where(hi == 1, trow - LO, trow)

        idx_vals = np.full(S_tot, PADROW, np.int64)
        idx_vals[pos] = tval
        ind = np.zeros((S_tot, P), np.float32)
        ind[pos, d & 127] = 1.0
        ind_w = ind.reshape(S_tot // P, P, P).transpose(1, 0, 2).copy()

        per_core.append(dict(
            idx=_wrap16(idx_vals),
            ind=_fp8(ind_w),
        ))
    return sched, per_core


def _host_inputs(inputs):
    import concourse.mybir as mybir
    bfnp = mybir.dt.np(mybir.dt.bfloat16)
    f32 = lambda a: np.asarray(a, np.float32)
    x = f32(inputs["x"])
    ei = np.asarray(inputs["edge_index"], np.int64)
    batch = np.asarray(inputs["batch"], np.int64)
    src, dst = ei[0], ei[1]

    deg = np.bincount(dst, minlength=N).astype(np.float32)
    invd_full = (1.0 / np.maximum(deg, 1.0)).astype(np.float32)

    sched, per_core = _build_schedule(src, dst, invd_full)

    xt = np.zeros((NT, F), bfnp)
    xb = _bf16(x)
    for r in range(NCORES):
        xt[r * SLAB:r * SLAB + NPC] = xb[r * NPC:(r + 1) * NPC]

    ident = np.eye(P, dtype=np.float32)

    # BN folding
    s_bn = f32(inputs["bn_gamma"]) / np.sqrt(f32(inputs["bn_rv"]) + EPS)
    t_bn = f32(inputs["bn_beta"]) - f32(inputs["bn_rm"]) * s_bn
    bns2 = s_bn.reshape(2, P).T.copy()     # [128, 2]
    bnt2 = t_bn.reshape(2, P).T.copy()

    shared = {
        "xt": xt, "ident": _bf16(ident),
        "bns2": bns2, "bnt2": bnt2,
        "l1w": _bf16(inputs["lin1_W"]), "l1b": f32(inputs["lin1_b"]),
        "l2w": _bf16(inputs["lin2_W"]), "l2b": f32(inputs["lin2_b"]),
    }
    for b in (0, 1):
        for nm in ("Wl1", "Wr1", "Wl2", "Wr2", "Wlin"):
            shared[f"b{b}_{nm}"] = _bf16(inputs[f"b{b}_{nm}"])
        for nm in ("b1", "b2", "blin"):
            shared[f"b{b}_{nm}"] = f32(inputs[f"b{b}_{nm}"])

    in_maps = []
    for c in range(NCORES):
        xoT = np.zeros((F, SLAB), bfnp)
        xoT[:, :NPC] = xb[c * NPC:(c + 1) * NPC].T
        ivb = np.zeros((P, SLAB), np.float32)
        ivb[:, :NPC] = invd_full[c * NPC:(c + 1) * NPC][None, :]
        pool_ind = np.zeros((NW, P, G), np.float32)
        bt = batch[c * NPC:(c + 1) * NPC]
        btp = np.full(NW * P, -1, np.int64)
        btp[:NPC] = bt
        btp2 = btp.reshape(NW, P)
        for wi in range(NW):
            vm = btp2[wi] >= 0
            pool_ind[wi, np.arange(P)[vm], btp2[wi][vm]] = 1.0
        im = dict(shared)
        im.update({
            "xoT": xoT, "poolind": _bf16(pool_ind), "invb": _bf16(ivb),
            "idx": per_core[c]["idx"], "ind": per_core[c]["ind"],
        })
        in_maps.append(im)
    return sched, in_maps


# ------------------------------------------------------------- bass program
def _build_program(sched, n_convs=4, debug_tables=False):
    import concourse.bass as bass
    import concourse.mybir as mybir
    import concourse.tile as tile
    from concourse import bacc
    from concourse import library_config
    from contextlib import ExitStack

    dt = mybir.dt
    DT = dt.float32
    BF = dt.bfloat16
    Alu = mybir.AluOpType
    Act = mybir.ActivationFunctionType

    nL, nH, gL, gH = (sched[k] for k in ("nL", "nH", "gL", "gH"))
    groups = sched["groups"]
    slot_base, hslot_base = sched["slot_base"], sched["hslot_base"]
    GMAX = sched["gmax"]
    TOTC = sched["tot_cols"]

    nc = bacc.Bacc("TRN2", debug=False, num_swdge_queues=4)

    # ---- parameters
    xt = nc.declare_dram_parameter("xt", [NT, F], BF, isOutput=False)
    xoT = nc.declare_dram_parameter("xoT", [F, SLAB], BF, isOutput=False)
    idxp = nc.declare_dram_parameter("idx", [P, TOTC * 8], dt.int16, isOutput=False)
    indp = nc.declare_dram_parameter("ind", [P, TOTC, P], dt.float8e4, isOutput=False)
    invbp = nc.declare_dram_parameter("invb", [P, SLAB], BF, isOutput=False)
    poolp = nc.declare_dram_parameter("poolind", [NW, P, G], BF, isOutput=False)
    identp = nc.declare_dram_parameter("ident", [P, P], BF, isOutput=False)
    wp = {}
    for b in (0, 1):
        for nm, shp, dty in (("Wl1", [F, HID], BF), ("Wr1", [F, HID], BF),
                             ("b1", [HID], DT),
                             ("Wl2", [HID, HID], BF), ("Wr2", [HID, HID], BF),
                             ("b2", [HID], DT),
                             ("Wlin", [2 * HID, HID], BF), ("blin", [HID], DT)):
            wp[f"b{b}_{nm}"] = nc.declare_dram_parameter(f"b{b}_{nm}", shp, dty, isOutput=False)
    bns2p = nc.declare_dram_parameter("bns2", [P, 2], DT, isOutput=False)
    bnt2p = nc.declare_dram_parameter("bnt2", [P, 2], DT, isOutput=False)
    l1wp = nc.declare_dram_parameter("l1w", [2 * HID, HID], BF, isOutput=False)
    l1bp = nc.declare_dram_parameter("l1b", [HID], DT, isOutput=False)
    l2wp = nc.declare_dram_parameter("l2w", [HID, C], BF, isOutput=False)
    l2bp = nc.declare_dram_parameter("l2b", [C], DT, isOutput=False)

    out = nc.declare_dram_parameter("out", [G, C], DT, isOutput=True)
    if debug_tables:
        dbgA = nc.declare_dram_parameter("dbgA", [NT, F], DT, isOutput=True)
        dbgB = nc.declare_dram_parameter("dbgB", [NT, F], DT, isOutput=True)

    with tile.TileContext(nc) as tc, ExitStack() as ctx:
        sb = ctx.enter_context(tc.tile_pool(name="sb", bufs=1))
        sb_feat = ctx.enter_context(tc.tile_pool(name="sb_feat", bufs=1))
        sb_g = ctx.enter_context(tc.tile_pool(name="sb_g", bufs=3))
        sb_idx = ctx.enter_context(tc.tile_pool(name="sb_idx", bufs=3))
        sb_ind = ctx.enter_context(tc.tile_pool(name="sb_ind", bufs=3))
        sb_ms = ctx.enter_context(tc.tile_pool(name="sb_ms", bufs=4))
        sb_pi = ctx.enter_context(tc.tile_pool(name="sb_pi", bufs=3))
        ps_agg = ctx.enter_context(tc.tile_pool(name="ps_agg", bufs=2, space="PSUM"))
        ps_mm = ctx.enter_context(tc.tile_pool(name="ps_mm", bufs=2, space="PSUM"))
        ps_tr = ctx.enter_context(tc.tile_pool(name="ps_tr", bufs=2, space="PSUM"))
        ps_pool = ctx.enter_context(tc.tile_pool(name="ps_pool", bufs=1, space="PSUM"))
        dram = ctx.enter_context(tc.tile_pool(name="dram", bufs=1, space="DRAM"))

        nc.gpsimd.load_library(library_config.mlp)

        # ---- constants into SBUF
        id_t = sb.tile([P, P], BF)
        nc.sync.dma_start(id_t[:], identp[:])
        invb_t = sb.tile([P, SLAB], BF)
        nc.sync.dma_start(invb_t[:], invbp[:])
        wt = {}
        for b in (0, 1):
            for nm in ("Wl1", "Wr1", "Wl2", "Wr2"):
                w_t = sb.tile([P, P], BF, name=f"w{b}{nm}")
                nc.sync.dma_start(w_t[:], wp[f"b{b}_{nm}"][:])
                wt[f"b{b}_{nm}"] = w_t
            wlin_t = sb.tile([P, 2, P], BF, name=f"w{b}lin")
            nc.sync.dma_start(wlin_t[:, 0, :], wp[f"b{b}_Wlin"][0:P, :])
            nc.sync.dma_start(wlin_t[:, 1, :], wp[f"b{b}_Wlin"][P:2 * P, :])
            wt[f"b{b}_Wlin"] = wlin_t
            for nm in ("b1", "b2", "blin"):
                b_t = sb.tile([P, 1], DT, name=f"b{b}{nm}")
                nc.sync.dma_start(b_t[:], wp[f"b{b}_{nm}"][:, None])
                wt[f"b{b}_{nm}"] = b_t
        bns_t = sb.tile([P, 2], DT)
        nc.sync.dma_start(bns_t[:], bns2p[:])
        bnt_t = sb.tile([P, 2], DT)
        nc.sync.dma_start(bnt_t[:], bnt2p[:])
        l1w_t = sb.tile([P, 2, P], BF)
        nc.sync.dma_start(l1w_t[:, 0, :], l1wp[0:P, :])
        nc.sync.dma_start(l1w_t[:, 1, :], l1wp[P:2 * P, :])
        l1b_t = sb.tile([P, 1], DT)
        nc.sync.dma_start(l1b_t[:], l1bp[:, None])
        l2w_t = sb.tile([P, C], BF)
        nc.sync.dma_start(l2w_t[:], l2wp[:])
        l2b_t = sb.tile([P, 1], DT)
        nc.sync.dma_start(l2b_t[0:C, :], l2bp[:, None])

        # feature-major activation buffers [128, SLAB] bf16
        featA = sb_feat.tile([P, SLAB], BF)
        featB = sb_feat.tile([P, SLAB], BF)
        featC = sb_feat.tile([P, SLAB], BF)
        nc.sync.dma_start(featA[:], xoT[:])

        zero_t = sb.tile([P, P], BF)
        nc.vector.memset(zero_t[:], 0.0)

        # DRAM scratch
        cA = dram.tile([SLAB, F], BF)
        cB = dram.tile([SLAB, F], BF)
        tabA = dram.tile([NT, F], BF, addr_space="Shared")
        tabB = dram.tile([NT, F], BF, addr_space="Shared")
        tabC = dram.tile([NT, F], BF, addr_space="Shared")
        pc_in = dram.tile([P, 2 * G], DT)
        pc_out = dram.tile([P, 2 * G], DT, addr_space="Shared")
        nc.sync.dma_start(cA[NPC:SLAB, :], zero_t[0:SLAB - NPC, :])
        nc.sync.dma_start(cB[NPC:SLAB, :], zero_t[0:SLAB - NPC, :])

        def conv(tab, in_feat, out_feat, Wl, Wr, bcol, contrib, jk=None):
            """One SAGE conv: out_feat[:, n] = relu(mean@Wl + in@Wr + b).
            If jk is given, also fuse the JumpingKnowledge linear + pooling:
            jk = (h1_feat, Wlin, blin_col, jk_contrib, hout, pool_ps)."""
            if not hasattr(conv, "qctr"):
                conv.qctr = 0
            for gi in groups:
                ws, colsL, cols, col0 = gi["ws"], gi["colsL"], gi["cols"], gi["col0"]
                g_t = sb_g.tile([P, GMAX, P], BF, name="g_t")
                ix = sb_idx.tile([P, GMAX * 8], dt.int16, name="ix")
                it3 = sb_ind.tile([P, GMAX, P], dt.float8e4, name="it3")
                nc.sync.dma_start(ix[:, 0:cols * 8],
                                  idxp[:, col0 * 8:(col0 + cols) * 8])
                nc.sync.dma_start(it3[:, 0:cols, :], indp[:, col0:col0 + cols, :])
                nlo = colsL * P
                nhi = (cols - colsL) * P
                nc.gpsimd.dma_gather(
                    g_t[:, 0:colsL, :], tab[0:LO], ix[:, 0:nlo // 16],
                    nlo, nlo, P, single_packet=True,
                    queue_num=conv.qctr % 4)
                conv.qctr += 1
                nc.gpsimd.dma_gather(
                    g_t[:, colsL:cols, :], tab[LO:NT], ix[:, nlo // 16:cols * 8],
                    nhi, nhi, P, single_packet=True,
                    queue_num=conv.qctr % 4)
                conv.qctr += 1

                for w in ws:
                    agg = ps_agg.tile([P, P], dt.float32, name="agg")
                    # tile columns of this window inside g_t
                    lo0 = (slot_base[w] // P) - col0
                    hi0 = (hslot_base[w] // P) - col0
                    jcols = ([lo0 + k for k in range(int(gL[w]))] +
                             [hi0 + k for k in range(int(gH[w]))])
                    njc = len(jcols)
                    for ji, j in enumerate(jcols):
                        nc.tensor.matmul(agg[:], g_t[:, j, :], it3[:, j, :],
                                         start=(ji == 0), stop=(ji == njc - 1))
                    mean_sb = sb_ms.tile([P, P], BF, name="mean_sb")
                    nc.vector.tensor_tensor(mean_sb[:], agg[:],
                                            invb_t[:, w * P:(w + 1) * P], Alu.mult)
                    h_ps = ps_mm.tile([P, P], dt.float32, name="h_ps", tag="mm")
                    nc.tensor.matmul(h_ps[:], Wl[:], mean_sb[:], start=True, stop=False)
                    nc.tensor.matmul(h_ps[:], Wr[:], in_feat[:, w * P:(w + 1) * P],
                                     start=False, stop=True)
                    nc.scalar.activation(out_feat[:, w * P:(w + 1) * P], h_ps[:],
                                         Act.Relu, bias=bcol[:], scale=1.0)
                    if contrib is not None:
                        rows = min(P, NPC - w * P)
                        hnm_ps = ps_tr.tile([P, P], BF, name="hnm_ps", tag="tr")
                        nc.tensor.transpose(hnm_ps[:], out_feat[:, w * P:(w + 1) * P], id_t[:])
                        hnm_sb = sb_ms.tile([P, P], BF, name="hnm_sb")
                        nc.scalar.copy(hnm_sb[:], hnm_ps[:])
                        nc.scalar.dma_start(contrib[w * P:w * P + rows, :], hnm_sb[0:rows, :])
                    if jk is not None:
                        h1f, Wlin, blc, jkcontrib, hout, pool_ps = jk
                        j_ps = ps_mm.tile([P, P], dt.float32, name="j_ps", tag="mm")
                        nc.tensor.matmul(j_ps[:], Wlin[:, 0, :], h1f[:, w * P:(w + 1) * P],
                                         start=True, stop=False)
                        nc.tensor.matmul(j_ps[:], Wlin[:, 1, :], out_feat[:, w * P:(w + 1) * P],
                                         start=False, stop=True)
                        nc.scalar.activation(hout[:, w * P:(w + 1) * P], j_ps[:],
                                             Act.Relu, bias=blc[:], scale=1.0)
                        jnm_ps = ps_tr.tile([P, P], BF, name="jnm_ps", tag="tr")
                        nc.tensor.transpose(jnm_ps[:], hout[:, w * P:(w + 1) * P], id_t[:])
                        jnm_sb = sb_ms.tile([P, P], BF, name="jnm_sb")
                        nc.scalar.copy(jnm_sb[:], jnm_ps[:])
                        if jkcontrib is not None:
                            rows = min(P, NPC - w * P)
                            nc.scalar.dma_start(jkcontrib[w * P:w * P + rows, :], jnm_sb[0:rows, :])
                        pind = sb_pi.tile([P, G], BF, name="pind")
                        nc.scalar.dma_start(pind[:], poolp[w])
                        nc.tensor.matmul(pool_ps[:], jnm_sb[:], pind[:],
                                         start=(w == 0), stop=(w == NW - 1))

        def allgather(contrib, tab):
            nc.gpsimd.collective_compute(
                "AllGather", Alu.bypass, ins=[contrib[:]], outs=[tab[:]],
                replica_groups=[list(range(NCORES))])

        # ---------------- block 0
        conv(xt, featA, featB, wt["b0_Wl1"], wt["b0_Wr1"], wt["b0_b1"], cA)   # h1
        allgather(cA, tabA)
        if n_convs >= 2:
            p0_ps = ps_pool.tile([P, G], dt.float32, name="p0_ps")
            conv(tabA, featB, featC, wt["b0_Wl2"], wt["b0_Wr2"], wt["b0_b2"], None,
                 jk=(featB, wt["b0_Wlin"], wt["b0_blin"], cB, featA, p0_ps))  # h2; h -> featA
            p0_sb = sb.tile([P, G], DT)
            nc.vector.tensor_copy(p0_sb[:], p0_ps[:])
            allgather(cB, tabB)
        if n_convs >= 3:
            conv(tabB, featA, featB, wt["b1_Wl1"], wt["b1_Wr1"], wt["b1_b1"], cA)  # h1'
            allgather(cA, tabC)
        if n_convs >= 4:
            p1_ps = ps_pool.tile([P, G], dt.float32, name="p1_ps")
            conv(tabC, featB, featC, wt["b1_Wl2"], wt["b1_Wr2"], wt["b1_b2"], None,
                 jk=(featB, wt["b1_Wlin"], wt["b1_blin"], None, featA, p1_ps))  # h2'
            p1_sb = sb.tile([P, G], DT)
            nc.vector.tensor_copy(p1_sb[:], p1_ps[:])

            # ---------------- pooling allreduce + head
            nc.sync.dma_start(pc_in[:, 0:G], p0_sb[:])
            nc.sync.dma_start(pc_in[:, G:2 * G], p1_sb[:])
            nc.gpsimd.collective_compute(
                "AllReduce", Alu.add, ins=[pc_in[:]], outs=[pc_out[:]],
                replica_groups=[list(range(NCORES))])
            pools_sb = sb.tile([P, 2 * G], DT)
            nc.sync.dma_start(pools_sb[:], pc_out[:])

            # BN (folded) per feature chunk -> bf16 for the head matmuls
            gbn = sb.tile([P, 2, G], BF)
            for k in range(2):
                nc.vector.tensor_scalar(gbn[:, k, :], pools_sb[:, k * G:(k + 1) * G],
                                        bns_t[:, k:k + 1], bnt_t[:, k:k + 1],
                                        Alu.mult, Alu.add)
            l1_ps = ps_mm.tile([P, G], dt.float32, name="l1_ps", tag="mm")
            for k in range(2):
                nc.tensor.matmul(l1_ps[:], l1w_t[:, k, :], gbn[:, k, :],
                                 start=(k == 0), stop=(k == 1))
            z1 = sb.tile([P, G], BF)
            nc.vector.tensor_scalar(z1[:], l1_ps[:], l1b_t[:], 0.0, Alu.add, Alu.max)
            l2_ps = ps_mm.tile([P, G], dt.float32, name="l2_ps", tag="mm")
            nc.tensor.matmul(l2_ps[0:C, :], l2w_t[:], z1[:], start=True, stop=True)
            z2 = sb.tile([P, G], DT)
            nc.vector.tensor_scalar(z2[0:C, :], l2_ps[0:C, :], l2b_t[0:C, :], None, Alu.add)

            # softmax over C (partition dim) -> transpose to [G, C] first
            zbf = sb.tile([P, G], BF)
            nc.vector.tensor_copy(zbf[0:C, :], z2[0:C, :])
            for half in range(2):
                zt_ps = ps_mm.tile([P, C], BF, name="zt_ps", tag="mm")
                nc.tensor.transpose(zt_ps[:, 0:C], zbf[0:C, half * P:(half + 1) * P], id_t[0:C, 0:C])
                znm = sb.tile([P, C], DT, name=f"znm{half}")
                nc.vector.tensor_copy(znm[:], zt_ps[:, 0:C])
                nmax = sb.tile([P, 1], DT, name=f"nmax{half}")
                nc.vector.tensor_reduce(nmax[:], znm[:], mybir.AxisListType.X, Alu.max, negate=True)
                e_t = sb.tile([P, C], DT, name=f"e_t{half}")
                nc.scalar.activation(e_t[:], znm[:], Act.Exp,
                                     bias=nmax[:], scale=1.0)
                ssum = sb.tile([P, 1], DT, name=f"ssum{half}")
                nc.vector.tensor_reduce(ssum[:], e_t[:], mybir.AxisListType.X, Alu.add)
                rcp = sb.tile([P, 1], DT, name=f"rcp{half}")
                nc.vector.reciprocal(rcp[:], ssum[:])
                sm = sb.tile([P, C], DT, name=f"sm{half}")
                nc.vector.tensor_scalar(sm[:], e_t[:], rcp[:], None, Alu.mult)
                nc.sync.dma_start(out[half * P:(half + 1) * P, :], sm[:])

        if debug_tables:
            for tabsrc, dbg in ((tabA, dbgA), (tabB, dbgB)):
                for r in range(NT // P):
                    st = sb_ms.tile([P, F], BF, name="dbg_st")
                    nc.sync.dma_start(st[:], tabsrc[r * P:(r + 1) * P, :])
                    st2 = sb_ms.tile([P, F], DT, name="dbg_st2")
                    nc.vector.tensor_copy(st2[:], st[:])
                    nc.sync.dma_start(dbg[r * P:(r + 1) * P, :], st2[:])

    nc.compile()
    return nc


# ------------------------------------------------------------------ runtime
def _install_profile_hook():
    try:
        from trn_agent_boot.trn_boot import _ntff_profile_via_ctypes
        hook = _ntff_profile_via_ctypes("/opt/axon/libaxon_pjrt.so")
        m = types.ModuleType("antenv.axon_hooks")
        m.get_axon_ntff_profile_hook = lambda: hook
        sys.modules.setdefault("antenv.axon_hooks", m)
    except Exception:
        pass


def kernel(**inputs):
    from concourse.bass_utils import run_bass_kernel_spmd

    n_convs = int(os.environ.get("KNC_CONVS", "4"))
    debug_tables = bool(int(os.environ.get("KDBG", "0")))
    trace = bool(int(os.environ.get("KTRACE", "0")))
    if trace:
        _install_profile_hook()

    sched, in_maps = _host_inputs(inputs)

    key = (n_convs, debug_tables, int(sched["tot_cols"]), int(sched["gmax"]))
    nc = _prog_cache.get(key)
    if nc is None:
        nc = _build_program(sched, n_convs=n_convs, debug_tables=debug_tables)
        _prog_cache[key] = nc

    res = run_bass_kernel_spmd(nc, in_maps, list(range(NCORES)), trace=trace)
    kernel.last_result = res
    out = res.results[0]["out"].astype(np.float32)
    return out


# revision 12
# speedup vs baseline: 3.0752x; 1.0077x over previous
"""GraphSAGE 2-block GNN (nn_BaselineModel_80607946211554) on 8 TRN2 NeuronCores.

Strategy: destination-node sharding, bf16 datapath. Each core owns 6250
contiguous nodes. Node-feature tables are replicated per-core in DRAM in a
slab layout (node n -> row (n//6250)*6272 + n%6250, 22 zero pad rows/slab).
Neighbor mean-aggregation per 128-dst window: dma_gather of bf16 source rows
(edges sorted by dst, host-preprocessed, sections padded to 128 only), then
for each 128-slot tile a DVE-generated indicator (iota==dloc)*inv_deg feeds a
PE matmul accumulating mean^T directly in PSUM. SAGE linears run
feature-major (weights stationary); PSUM->SBUF copies and bias+ReLU run on
the Activation engine. Intermediate tables rebuilt via bf16 AllGather; graph
pooling is a one-hot matmul; the MLP head + softmax is replicated per core.

Self-contained: hardcodes all shapes for the fixed problem instance.
"""
import os
import sys
import types
import numpy as np

N = 50000
E = 1600000
G = 256
F = 128
HID = 128
C = 10
NCORES = 8
NPC = N // NCORES            # 6250 nodes per core
SLAB = 6272                  # slab rows (6250 + 22 zero pad)
NT = NCORES * SLAB           # 50176 table rows
R1 = 3200                    # chunk1 local rows [0,3200) = windows 0-24
C1 = 3328                    # chunk1 rows per core (3200 + 128 zero pad)
C2 = 3072                    # chunk2 rows per core (3050 + 22 zero pad)
NT1 = NCORES * C1            # 26624 (< 32768: int16 ok)
NT2 = NCORES * C2            # 24576
PAD0 = R1                    # zero row in chunk1 view (core 0 pad)
PAD1 = 3050                  # zero row in chunk2 view (core 0 pad)
P = 128
NW1 = R1 // P                # 25 windows in chunk1
NW = (NPC + P - 1) // P      # 49 dst windows per core
EPS = 1e-5
GCAP = 96                    # max 128-slot tiles per gather group

_prog_cache = {}


def _bf16(a):
    import concourse.mybir as mybir
    return np.asarray(a, np.float32).astype(mybir.dt.np(mybir.dt.bfloat16))


def _fp8(a):
    import concourse.mybir as mybir
    return np.asarray(a, np.float32).astype(mybir.dt.np(mybir.dt.float8e4))


def _wrap16(vals):
    """int64 slot values (len mult of 16) -> [128, n/16] int16 wrapped."""
    n = len(vals)
    arr = vals.reshape(n // 16, 16).T.astype(np.int16)   # [16, n/16]
    return np.tile(arr, (8, 1))                           # [128, n/16]


def _wrap128(vals):
    """[S] -> [128, S/128]: slot s -> [s%128, s//128]."""
    return vals.reshape(-1, 128).T.copy()


def _build_schedule(src, dst, invd_full):
    """Static shared schedule + per-core gather index / metadata arrays."""
    core_edges = []
    CL = np.zeros((NCORES, NW), np.int64)
    CH = np.zeros((NCORES, NW), np.int64)
    for c in range(NCORES):
        m = (dst >= c * NPC) & (dst < (c + 1) * NPC)
        s = src[m].astype(np.int64)
        d = (dst[m] - c * NPC).astype(np.int64)
        sq = s // NPC
        sr = s % NPC
        hi = (sr >= R1).astype(np.int64)
        w = d >> 7
        order = np.lexsort((d, hi, w))
        s, d, hi, w, sq, sr = s[order], d[order], hi[order], w[order], sq[order], sr[order]
        core_edges.append((s, d, hi, w, sq, sr))
        cnt = np.bincount(w * 2 + hi, minlength=NW * 2).reshape(NW, 2)
        CL[c], CH[c] = cnt[:, 0], cnt[:, 1]

    nL = np.maximum(((CL.max(0) + 127) // P) * P, P)
    nH = np.maximum(((CH.max(0) + 127) // P) * P, P)
    gL, gH = nL // P, nH // P                     # tiles per section
    ngrp = gL + gH

    # pack consecutive windows into gather groups of <= GCAP tiles
    groups = []
    cur = []
    cur_cols = 0
    for w in range(NW):
        if cur and cur_cols + ngrp[w] > GCAP:
            groups.append(cur)
            cur, cur_cols = [], 0
        cur.append(w)
        cur_cols += int(ngrp[w])
    if cur:
        groups.append(cur)

    # layouts
    ginfo = []       # per group: dict
    slot_base = np.zeros(NW, np.int64)   # base slot of lo section of window
    hslot_base = np.zeros(NW, np.int64)  # base slot of hi section of window
    col0 = 0         # running tile column over all groups
    for ws in groups:
        colsL = int(gL[ws].sum())
        cols = int(ngrp[ws].sum())
        off = 0
        for w in ws:
            slot_base[w] = (col0 + off) * P
            off += int(gL[w])
        for w in ws:
            hslot_base[w] = (col0 + off) * P
            off += int(gH[w])
        ginfo.append(dict(ws=ws, colsL=colsL, cols=cols, col0=col0))
        col0 += cols
    tot_cols = col0
    S_tot = tot_cols * P

    sched = dict(nL=nL, nH=nH, gL=gL, gH=gH, ngrp=ngrp, groups=ginfo,
                 slot_base=slot_base, hslot_base=hslot_base,
                 tot_cols=tot_cols, S_tot=S_tot,
                 gmax=max(g["cols"] for g in ginfo))

    sched["nLw"] = nL
    sched["nHw"] = nH
    per_core = []
    for c in range(NCORES):
        s, d, hi, w, sq, sr = core_edges[c]
        # rank within (w, hi) section
        key = w * 2 + hi
        if len(key):
            grp_change = np.r_[True, key[1:] != key[:-1]]
            first_pos = np.flatnonzero(grp_change)
            gidx = np.cumsum(grp_change) - 1
            rank = np.arange(len(d)) - first_pos[gidx]
        else:
            rank = np.zeros(0, np.int64)
        base = np.where(hi == 1, hslot_base[w], slot_base[w])
        pos = base + rank

        tval = np.where(hi == 1, sq * C2 + (sr - R1), sq * C1 + sr)

        idx_vals = np.full(S_tot, PAD0, np.int64)
        for wi in range(NW):
            h0 = hslot_base[wi]
            idx_vals[h0:h0 + nH[wi]] = PAD1
        idx_vals[pos] = tval
        ind = np.zeros((S_tot, P), np.float32)
        ind[pos, d & 127] = 1.0
        ind_w = ind.reshape(S_tot // P, P, P).transpose(1, 0, 2).copy()

        per_core.append(dict(
            idx=_wrap16(idx_vals),
            ind=_fp8(ind_w),
        ))
    return sched, per_core


def _host_inputs(inputs):
    import concourse.mybir as mybir
    bfnp = mybir.dt.np(mybir.dt.bfloat16)
    f32 = lambda a: np.asarray(a, np.float32)
    x = f32(inputs["x"])
    ei = np.asarray(inputs["edge_index"], np.int64)
    batch = np.asarray(inputs["batch"], np.int64)
    src, dst = ei[0], ei[1]

    deg = np.bincount(dst, minlength=N).astype(np.float32)
    invd_full = (1.0 / np.maximum(deg, 1.0)).astype(np.float32)

    sched, per_core = _build_schedule(src, dst, invd_full)

    xb = _bf16(x)
    xt1 = np.zeros((NT1, F), bfnp)
    xt2 = np.zeros((NT2, F), bfnp)
    for r in range(NCORES):
        xt1[r * C1:r * C1 + R1] = xb[r * NPC:r * NPC + R1]
        xt2[r * C2:r * C2 + (NPC - R1)] = xb[r * NPC + R1:(r + 1) * NPC]

    ident = np.eye(P, dtype=np.float32)

    # BN folding
    s_bn = f32(inputs["bn_gamma"]) / np.sqrt(f32(inputs["bn_rv"]) + EPS)
    t_bn = f32(inputs["bn_beta"]) - f32(inputs["bn_rm"]) * s_bn
    bns2 = s_bn.reshape(2, P).T.copy()     # [128, 2]
    bnt2 = t_bn.reshape(2, P).T.copy()

    shared = {
        "xt1": xt1, "xt2": xt2, "ident": _bf16(ident),
        "bns2": bns2, "bnt2": bnt2,
        "l1w": _bf16(inputs["lin1_W"]), "l1b": f32(inputs["lin1_b"]),
        "l2w": _bf16(inputs["lin2_W"]), "l2b": f32(inputs["lin2_b"]),
    }
    for b in (0, 1):
        for nm in ("Wl1", "Wr1", "Wl2", "Wr2", "Wlin"):
            shared[f"b{b}_{nm}"] = _bf16(inputs[f"b{b}_{nm}"])
        for nm in ("b1", "b2", "blin"):
            shared[f"b{b}_{nm}"] = f32(inputs[f"b{b}_{nm}"])

    in_maps = []
    for c in range(NCORES):
        xoT = np.zeros((F, SLAB), bfnp)
        xoT[:, :NPC] = xb[c * NPC:(c + 1) * NPC].T
        ivb = np.zeros((P, SLAB), np.float32)
        ivb[:, :NPC] = invd_full[c * NPC:(c + 1) * NPC][None, :]
        pool_ind = np.zeros((NW, P, G), np.float32)
        bt = batch[c * NPC:(c + 1) * NPC]
        btp = np.full(NW * P, -1, np.int64)
        btp[:NPC] = bt
        btp2 = btp.reshape(NW, P)
        for wi in range(NW):
            vm = btp2[wi] >= 0
            pool_ind[wi, np.arange(P)[vm], btp2[wi][vm]] = 1.0
        im = dict(shared)
        im.update({
            "xoT": xoT, "poolind": _bf16(pool_ind), "invb": _bf16(ivb),
            "idx": per_core[c]["idx"], "ind": per_core[c]["ind"],
        })
        in_maps.append(im)
    return sched, in_maps


# ------------------------------------------------------------- bass program
def _build_program(sched, n_convs=4, debug_tables=False):
    import concourse.bass as bass
    import concourse.mybir as mybir
    import concourse.tile as tile
    from concourse import bacc
    from concourse import library_config
    from contextlib import ExitStack

    dt = mybir.dt
    DT = dt.float32
    BF = dt.bfloat16
    Alu = mybir.AluOpType
    Act = mybir.ActivationFunctionType

    nL, nH, gL, gH = (sched[k] for k in ("nL", "nH", "gL", "gH"))
    groups = sched["groups"]
    slot_base, hslot_base = sched["slot_base"], sched["hslot_base"]
    GMAX = sched["gmax"]
    TOTC = sched["tot_cols"]

    nc = bacc.Bacc("TRN2", debug=False, num_swdge_queues=4)

    # ---- parameters
    xt1 = nc.declare_dram_parameter("xt1", [NT1, F], BF, isOutput=False)
    xt2 = nc.declare_dram_parameter("xt2", [NT2, F], BF, isOutput=False)
    xoT = nc.declare_dram_parameter("xoT", [F, SLAB], BF, isOutput=False)
    idxp = nc.declare_dram_parameter("idx", [P, TOTC * 8], dt.int16, isOutput=False)
    indp = nc.declare_dram_parameter("ind", [P, TOTC, P], dt.float8e4, isOutput=False)
    invbp = nc.declare_dram_parameter("invb", [P, SLAB], BF, isOutput=False)
    poolp = nc.declare_dram_parameter("poolind", [NW, P, G], BF, isOutput=False)
    identp = nc.declare_dram_parameter("ident", [P, P], BF, isOutput=False)
    wp = {}
    for b in (0, 1):
        for nm, shp, dty in (("Wl1", [F, HID], BF), ("Wr1", [F, HID], BF),
                             ("b1", [HID], DT),
                             ("Wl2", [HID, HID], BF), ("Wr2", [HID, HID], BF),
                             ("b2", [HID], DT),
                             ("Wlin", [2 * HID, HID], BF), ("blin", [HID], DT)):
            wp[f"b{b}_{nm}"] = nc.declare_dram_parameter(f"b{b}_{nm}", shp, dty, isOutput=False)
    bns2p = nc.declare_dram_parameter("bns2", [P, 2], DT, isOutput=False)
    bnt2p = nc.declare_dram_parameter("bnt2", [P, 2], DT, isOutput=False)
    l1wp = nc.declare_dram_parameter("l1w", [2 * HID, HID], BF, isOutput=False)
    l1bp = nc.declare_dram_parameter("l1b", [HID], DT, isOutput=False)
    l2wp = nc.declare_dram_parameter("l2w", [HID, C], BF, isOutput=False)
    l2bp = nc.declare_dram_parameter("l2b", [C], DT, isOutput=False)

    out = nc.declare_dram_parameter("out", [G, C], DT, isOutput=True)
    if debug_tables:
        dbgA = nc.declare_dram_parameter("dbgA", [NT1 + NT2, F], DT, isOutput=True)
        dbgB = nc.declare_dram_parameter("dbgB", [NT1 + NT2, F], DT, isOutput=True)

    with tile.TileContext(nc) as tc, ExitStack() as ctx:
        sb = ctx.enter_context(tc.tile_pool(name="sb", bufs=1))
        sb_feat = ctx.enter_context(tc.tile_pool(name="sb_feat", bufs=1))
        sb_g = ctx.enter_context(tc.tile_pool(name="sb_g", bufs=3))
        sb_idx = ctx.enter_context(tc.tile_pool(name="sb_idx", bufs=3))
        sb_ind = ctx.enter_context(tc.tile_pool(name="sb_ind", bufs=3))
        sb_ms = ctx.enter_context(tc.tile_pool(name="sb_ms", bufs=4))
        sb_pi = ctx.enter_context(tc.tile_pool(name="sb_pi", bufs=3))
        ps_agg = ctx.enter_context(tc.tile_pool(name="ps_agg", bufs=2, space="PSUM"))
        ps_mm = ctx.enter_context(tc.tile_pool(name="ps_mm", bufs=2, space="PSUM"))
        ps_tr = ctx.enter_context(tc.tile_pool(name="ps_tr", bufs=2, space="PSUM"))
        ps_pool = ctx.enter_context(tc.tile_pool(name="ps_pool", bufs=1, space="PSUM"))
        dram = ctx.enter_context(tc.tile_pool(name="dram", bufs=1, space="DRAM"))

        nc.gpsimd.load_library(library_config.mlp)

        # ---- constants into SBUF
        id_t = sb.tile([P, P], BF)
        nc.sync.dma_start(id_t[:], identp[:])
        invb_t = sb.tile([P, SLAB], BF)
        nc.sync.dma_start(invb_t[:], invbp[:])
        wt = {}
        for b in (0, 1):
            for nm in ("Wl1", "Wr1", "Wl2", "Wr2"):
                w_t = sb.tile([P, P], BF, name=f"w{b}{nm}")
                nc.sync.dma_start(w_t[:], wp[f"b{b}_{nm}"][:])
                wt[f"b{b}_{nm}"] = w_t
            wlin_t = sb.tile([P, 2, P], BF, name=f"w{b}lin")
            nc.sync.dma_start(wlin_t[:, 0, :], wp[f"b{b}_Wlin"][0:P, :])
            nc.sync.dma_start(wlin_t[:, 1, :], wp[f"b{b}_Wlin"][P:2 * P, :])
            wt[f"b{b}_Wlin"] = wlin_t
            for nm in ("b1", "b2", "blin"):
                b_t = sb.tile([P, 1], DT, name=f"b{b}{nm}")
                nc.sync.dma_start(b_t[:], wp[f"b{b}_{nm}"][:, None])
                wt[f"b{b}_{nm}"] = b_t
        bns_t = sb.tile([P, 2], DT)
        nc.sync.dma_start(bns_t[:], bns2p[:])
        bnt_t = sb.tile([P, 2], DT)
        nc.sync.dma_start(bnt_t[:], bnt2p[:])
        l1w_t = sb.tile([P, 2, P], BF)
        nc.sync.dma_start(l1w_t[:, 0, :], l1wp[0:P, :])
        nc.sync.dma_start(l1w_t[:, 1, :], l1wp[P:2 * P, :])
        l1b_t = sb.tile([P, 1], DT)
        nc.sync.dma_start(l1b_t[:], l1bp[:, None])
        l2w_t = sb.tile([P, C], BF)
        nc.sync.dma_start(l2w_t[:], l2wp[:])
        l2b_t = sb.tile([P, 1], DT)
        nc.sync.dma_start(l2b_t[0:C, :], l2bp[:, None])

        # feature-major activation buffers [128, SLAB] bf16
        featA = sb_feat.tile([P, SLAB], BF)
        featB = sb_feat.tile([P, SLAB], BF)
        featC = sb_feat.tile([P, SLAB], BF)
        nc.sync.dma_start(featA[:], xoT[:])

        zero_t = sb.tile([P, P], BF)
        nc.vector.memset(zero_t[:], 0.0)

        # DRAM scratch (two-chunk tables: AG of chunk1 overlaps late windows)
        cA1 = dram.tile([C1, F], BF)
        cA2 = dram.tile([C2, F], BF)
        cB1 = dram.tile([C1, F], BF)
        cB2 = dram.tile([C2, F], BF)
        tabA1 = dram.tile([NT1, F], BF, addr_space="Shared")
        tabA2 = dram.tile([NT2, F], BF, addr_space="Shared")
        tabB1 = dram.tile([NT1, F], BF, addr_space="Shared")
        tabB2 = dram.tile([NT2, F], BF, addr_space="Shared")
        tabC1 = dram.tile([NT1, F], BF, addr_space="Shared")
        tabC2 = dram.tile([NT2, F], BF, addr_space="Shared")
        pc_in = dram.tile([P, 2 * G], DT)
        pc_out = dram.tile([P, 2 * G], DT, addr_space="Shared")
        nc.sync.dma_start(cA1[R1:C1, :], zero_t[0:C1 - R1, :])
        nc.sync.dma_start(cA2[PAD1:C2, :], zero_t[0:C2 - PAD1, :])
        nc.sync.dma_start(cB1[R1:C1, :], zero_t[0:C1 - R1, :])
        nc.sync.dma_start(cB2[PAD1:C2, :], zero_t[0:C2 - PAD1, :])

        def conv(tab1, tab2, in_feat, out_feat, Wl, Wr, bcol, contrib, jk=None,
                 ag1=None):
            """One SAGE conv: out_feat[:, n] = relu(mean@Wl + in@Wr + b).
            contrib = (c1, c2) chunked node-major output buffers or None.
            jk = (h1_feat, Wlin, blin_col, jk_contrib_pair, hout, pool_ps).
            ag1 = (src1, dst1) chunk-1 AllGather fired mid-conv."""
            if not hasattr(conv, "qctr"):
                conv.qctr = 0
            ag1_fired = False
            for gi in groups:
                ws, colsL, cols, col0 = gi["ws"], gi["colsL"], gi["cols"], gi["col0"]
                g_t = sb_g.tile([P, GMAX, P], BF, name="g_t")
                ix = sb_idx.tile([P, GMAX * 8], dt.int16, name="ix")
                it3 = sb_ind.tile([P, GMAX, P], dt.float8e4, name="it3")
                nc.sync.dma_start(ix[:, 0:cols * 8],
                                  idxp[:, col0 * 8:(col0 + cols) * 8])
                nc.sync.dma_start(it3[:, 0:cols, :], indp[:, col0:col0 + cols, :])
                nlo = colsL * P
                nhi = (cols - colsL) * P
                nc.gpsimd.dma_gather(
                    g_t[:, 0:colsL, :], tab[0:LO], ix[:, 0:nlo // 16],
                    nlo, nlo, P, single_packet=True,
                    queue_num=conv.qctr % 4)
                conv.qctr += 1
                nc.gpsimd.dma_gather(
                    g_t[:, colsL:cols, :], tab[LO:NT], ix[:, nlo // 16:cols * 8],
                    nhi, nhi, P, single_packet=True,
                    queue_num=conv.qctr % 4)
                conv.qctr += 1

                for w in ws:
                    agg = ps_agg.tile([P, P], dt.float32, name="agg")
                    # tile columns of this window inside g_t
                    lo0 = (slot_base[w] // P) - col0
                    hi0 = (hslot_base[w] // P) - col0
                    jcols = ([lo0 + k for k in range(int(gL[w]))] +
                             [hi0 + k for k in range(int(gH[w]))])
                    njc = len(jcols)
                    for ji, j in enumerate(jcols):
                        nc.tensor.matmul(agg[:], g_t[:, j, :], it3[:, j, :],
                                         start=(ji == 0), stop=(ji == njc - 1))
                    mean_sb = sb_ms.tile([P, P], BF, name="mean_sb")
                    nc.vector.tensor_tensor(mean_sb[:], agg[:],
                                            invb_t[:, w * P:(w + 1) * P], Alu.mult)
                    h_ps = ps_mm.tile([P, P], dt.float32, name="h_ps", tag="mm")
                    nc.tensor.matmul(h_ps[:], Wl[:], mean_sb[:], start=True, stop=False)
                    nc.tensor.matmul(h_ps[:], Wr[:], in_feat[:, w * P:(w + 1) * P],
                                     start=False, stop=True)
                    nc.scalar.activation(out_feat[:, w * P:(w + 1) * P], h_ps[:],
                                         Act.Relu, bias=bcol[:], scale=1.0)
                    if contrib is not None:
                        rows = min(P, NPC - w * P)
                        hnm_ps = ps_tr.tile([P, P], BF, name="hnm_ps", tag="tr")
                        nc.tensor.transpose(hnm_ps[:], out_feat[:, w * P:(w + 1) * P], id_t[:])
                        hnm_sb = sb_ms.tile([P, P], BF, name="hnm_sb")
                        nc.scalar.copy(hnm_sb[:], hnm_ps[:])
                        if w < NW1:
                            nc.scalar.dma_start(contrib[0][w * P:w * P + rows, :], hnm_sb[0:rows, :])
                        else:
                            r0 = (w - NW1) * P
                            nc.scalar.dma_start(contrib[1][r0:r0 + rows, :], hnm_sb[0:rows, :])
                    if jk is not None:
                        h1f, Wlin, blc, jkcontrib, hout, pool_ps = jk
                        j_ps = ps_mm.tile([P, P], dt.float32, name="j_ps", tag="mm")
                        nc.tensor.matmul(j_ps[:], Wlin[:, 0, :], h1f[:, w * P:(w + 1) * P],
                                         start=True, stop=False)
                        nc.tensor.matmul(j_ps[:], Wlin[:, 1, :], out_feat[:, w * P:(w + 1) * P],
                                         start=False, stop=True)
                        nc.scalar.activation(hout[:, w * P:(w + 1) * P], j_ps[:],
                                             Act.Relu, bias=blc[:], scale=1.0)
                        jnm_ps = ps_tr.tile([P, P], BF, name="jnm_ps", tag="tr")
                        nc.tensor.transpose(jnm_ps[:], hout[:, w * P:(w + 1) * P], id_t[:])
                        jnm_sb = sb_ms.tile([P, P], BF, name="jnm_sb")
                        nc.scalar.copy(jnm_sb[:], jnm_ps[:])
                        if jkcontrib is not None:
                            rows = min(P, NPC - w * P)
                            if w < NW1:
                                nc.scalar.dma_start(jkcontrib[0][w * P:w * P + rows, :], jnm_sb[0:rows, :])
                            else:
                                r0 = (w - NW1) * P
                                nc.scalar.dma_start(jkcontrib[1][r0:r0 + rows, :], jnm_sb[0:rows, :])
                        pind = sb_pi.tile([P, G], BF, name="pind")
                        nc.scalar.dma_start(pind[:], poolp[w])
                        nc.tensor.matmul(pool_ps[:], jnm_sb[:], pind[:],
                                         start=(w == 0), stop=(w == NW - 1))
                if ag1 is not None and not ag1_fired and ws[-1] >= 36:
                    allgather(ag1[0], ag1[1])
                    ag1_fired = True

        def allgather(contrib, tab):
            nc.gpsimd.collective_compute(
                "AllGather", Alu.bypass, ins=[contrib[:]], outs=[tab[:]],
                replica_groups=[list(range(NCORES))])

        # ---------------- block 0
        conv(xt1, xt2, featA, featB, wt["b0_Wl1"], wt["b0_Wr1"], wt["b0_b1"],
             (cA1, cA2), ag1=(cA1, tabA1))   # h1
        allgather(cA2, tabA2)
        if n_convs >= 2:
            p0_ps = ps_pool.tile([P, G], dt.float32, name="p0_ps")
            conv(tabA1, tabA2, featB, featC, wt["b0_Wl2"], wt["b0_Wr2"], wt["b0_b2"], None,
                 jk=(featB, wt["b0_Wlin"], wt["b0_blin"], (cB1, cB2), featA, p0_ps),
                 ag1=(cB1, tabB1))  # h2; h -> featA
            p0_sb = sb.tile([P, G], DT)
            nc.vector.tensor_copy(p0_sb[:], p0_ps[:])
            allgather(cB2, tabB2)
        if n_convs >= 3:
            conv(tabB1, tabB2, featA, featB, wt["b1_Wl1"], wt["b1_Wr1"], wt["b1_b1"],
                 (cA1, cA2), ag1=(cA1, tabC1))  # h1'
            allgather(cA2, tabC2)
        if n_convs >= 4:
            p1_ps = ps_pool.tile([P, G], dt.float32, name="p1_ps")
            conv(tabC1, tabC2, featB, featC, wt["b1_Wl2"], wt["b1_Wr2"], wt["b1_b2"], None,
                 jk=(featB, wt["b1_Wlin"], wt["b1_blin"], None, featA, p1_ps))  # h2'
            p1_sb = sb.tile([P, G], DT)
            nc.vector.tensor_copy(p1_sb[:], p1_ps[:])

            # ---------------- pooling allreduce + head
            nc.sync.dma_start(pc_in[:, 0:G], p0_sb[:])
            nc.sync.dma_start(pc_in[:, G:2 * G], p1_sb[:])
            nc.gpsimd.collective_compute(
                "AllReduce", Alu.add, ins=[pc_in[:]], outs=[pc_out[:]],
                replica_groups=[list(range(NCORES))])
            pools_sb = sb.tile([P, 2 * G], DT)
            nc.sync.dma_start(pools_sb[:], pc_out[:])

            # BN (folded) per feature chunk -> bf16 for the head matmuls
            gbn = sb.tile([P, 2, G], BF)
            for k in range(2):
                nc.vector.tensor_scalar(gbn[:, k, :], pools_sb[:, k * G:(k + 1) * G],
                                        bns_t[:, k:k + 1], bnt_t[:, k:k + 1],
                                        Alu.mult, Alu.add)
            l1_ps = ps_mm.tile([P, G], dt.float32, name="l1_ps", tag="mm")
            for k in range(2):
                nc.tensor.matmul(l1_ps[:], l1w_t[:, k, :], gbn[:, k, :],
                                 start=(k == 0), stop=(k == 1))
            z1 = sb.tile([P, G], BF)
            nc.vector.tensor_scalar(z1[:], l1_ps[:], l1b_t[:], 0.0, Alu.add, Alu.max)
            l2_ps = ps_mm.tile([P, G], dt.float32, name="l2_ps", tag="mm")
            nc.tensor.matmul(l2_ps[0:C, :], l2w_t[:], z1[:], start=True, stop=True)
            z2 = sb.tile([P, G], DT)
            nc.vector.tensor_scalar(z2[0:C, :], l2_ps[0:C, :], l2b_t[0:C, :], None, Alu.add)

            # softmax over C (partition dim) -> transpose to [G, C] first
            zbf = sb.tile([P, G], BF)
            nc.vector.tensor_copy(zbf[0:C, :], z2[0:C, :])
            for half in range(2):
                zt_ps = ps_mm.tile([P, C], BF, name="zt_ps", tag="mm")
                nc.tensor.transpose(zt_ps[:, 0:C], zbf[0:C, half * P:(half + 1) * P], id_t[0:C, 0:C])
                znm = sb.tile([P, C], DT, name=f"znm{half}")
                nc.vector.tensor_copy(znm[:], zt_ps[:, 0:C])
                nmax = sb.tile([P, 1], DT, name=f"nmax{half}")
                nc.vector.tensor_reduce(nmax[:], znm[:], mybir.AxisListType.X, Alu.max, negate=True)
                e_t = sb.tile([P, C], DT, name=f"e_t{half}")
                nc.scalar.activation(e_t[:], znm[:], Act.Exp,
                                     bias=nmax[:], scale=1.0)
                ssum = sb.tile([P, 1], DT, name=f"ssum{half}")
                nc.vector.tensor_reduce(ssum[:], e_t[:], mybir.AxisListType.X, Alu.add)
                rcp = sb.tile([P, 1], DT, name=f"rcp{half}")
                nc.vector.reciprocal(rcp[:], ssum[:])
                sm = sb.tile([P, C], DT, name=f"sm{half}")
                nc.vector.tensor_scalar(sm[:], e_t[:], rcp[:], None, Alu.mult)
                nc.sync.dma_start(out[half * P:(half + 1) * P, :], sm[:])

        if debug_tables:
            for t1, t2, dbg in ((tabA1, tabA2, dbgA), (tabB1, tabB2, dbgB)):
                for r in range(NT1 // P):
                    st = sb_ms.tile([P, F], BF, name="dbg_st")
                    nc.sync.dma_start(st[:], t1[r * P:(r + 1) * P, :])
                    st2 = sb_ms.tile([P, F], DT, name="dbg_st2")
                    nc.vector.tensor_copy(st2[:], st[:])
                    nc.sync.dma_start(dbg[r * P:(r + 1) * P, :], st2[:])
                for r in range(NT2 // P):
                    st = sb_ms.tile([P, F], BF, name="dbg_st")
                    nc.sync.dma_start(st[:], t2[r * P:(r + 1) * P, :])
                    st2 = sb_ms.tile([P, F], DT, name="dbg_st2")
                    nc.vector.tensor_copy(st2[:], st[:])
                    nc.sync.dma_start(dbg[NT1 + r * P:NT1 + (r + 1) * P, :], st2[:])

    nc.compile()
    return nc


# ------------------------------------------------------------------ runtime
def _install_profile_hook():
    try:
        from trn_agent_boot.trn_boot import _ntff_profile_via_ctypes
        hook = _ntff_profile_via_ctypes("/opt/axon/libaxon_pjrt.so")
        m = types.ModuleType("antenv.axon_hooks")
        m.get_axon_ntff_profile_hook = lambda: hook
        sys.modules.setdefault("antenv.axon_hooks", m)
    except Exception:
        pass


def kernel(**inputs):
    from concourse.bass_utils import run_bass_kernel_spmd

    n_convs = int(os.environ.get("KNC_CONVS", "4"))
    debug_tables = bool(int(os.environ.get("KDBG", "0")))
    trace = bool(int(os.environ.get("KTRACE", "0")))
    if trace:
        _install_profile_hook()

    sched, in_maps = _host_inputs(inputs)

    key = (n_convs, debug_tables, int(sched["tot_cols"]), int(sched["gmax"]))
    nc = _prog_cache.get(key)
    if nc is None:
        nc = _build_program(sched, n_convs=n_convs, debug_tables=debug_tables)
        _prog_cache[key] = nc

    res = run_bass_kernel_spmd(nc, in_maps, list(range(NCORES)), trace=trace)
    kernel.last_result = res
    out = res.results[0]["out"].astype(np.float32)
    return out
